# revision 1
# baseline (speedup 1.0000x reference)
"""MinamoTopoModel GAT kernel: host preprocessing + Bass builder.

Self-contained logic module; kernel.py inlines/imports this during dev.
Design (per 8-core SPMD, dst-sharded):
  L1: cnt-histogram trick (host) -> per-group matmuls, no edge gathers.
  L2/L3: per-tile (128-edge) indirect DMA gathers of node records +
         S-matrix (iota-compare) PSUM scatter matmuls, segment softmax
         without max-subtraction, self-loops handled per-group directly.
  Two AllGathers publish per-shard node records between layers.
  Graph pooling -> per-core [50,17] partials; final FC on host.
"""
import numpy as np
import concourse.bacc as bacc
import concourse.bass as bass
import concourse.mybir as mybir
import concourse.tile as tile

F32 = mybir.dt.float32
I32 = mybir.dt.int32
AX = mybir.AxisListType
ALU = mybir.AluOpType
ACT = mybir.ActivationFunctionType
EPS = 1e-5


def host_prep(inputs, N, E, G, NC, TILE=32, EMB=16):
    H1, C1, H2, C2, H3, C3 = 8, 64, 4, 128, 1, 16
    x = np.asarray(inputs['x']).astype(np.int64)
    ei = np.asarray(inputs['edge_index']).astype(np.int64)
    batch = np.asarray(inputs['batch']).astype(np.int64)
    emb = np.asarray(inputs['emb'], np.float32)
    W1 = np.asarray(inputs['W1'], np.float32)
    as1 = np.asarray(inputs['a_src1'], np.float32); ad1 = np.asarray(inputs['a_dst1'], np.float32)
    b1 = np.asarray(inputs['b1'], np.float32)
    g1 = np.asarray(inputs['g1'], np.float32); be1 = np.asarray(inputs['be1'], np.float32)
    W2 = np.asarray(inputs['W2'], np.float32)
    as2 = np.asarray(inputs['a_src2'], np.float32); ad2 = np.asarray(inputs['a_dst2'], np.float32)
    b2 = np.asarray(inputs['b2'], np.float32)
    g2 = np.asarray(inputs['g2'], np.float32); be2 = np.asarray(inputs['be2'], np.float32)
    W3 = np.asarray(inputs['W3'], np.float32)
    as3 = np.asarray(inputs['a_src3'], np.float32); ad3 = np.asarray(inputs['a_dst3'], np.float32)
    b3 = np.asarray(inputs['b3'], np.float32)
    g3 = np.asarray(inputs['g3'], np.float32); be3 = np.asarray(inputs['be3'], np.float32)

    NPC = N // NC                      # nodes per core (exact: 50000/8=6250)
    NG = (NPC + 127) // 128            # groups per core (49)
    NPCP = NG * 128                    # padded nodes per core (6272)

    # ---- L1 tables (cnt trick) ----
    z1 = emb @ W1                                     # [32, 512]
    z1h = z1.reshape(TILE, H1, C1)
    al1t = np.einsum('thc,hc->th', z1h, as1)          # [32,8]
    ar1t = np.einsum('thc,hc->th', z1h, ad1)
    # E_tab[xd, h, t] = exp(lrelu(al1t[t,h] + ar1t[xd,h]))
    ee = al1t.T[None, :, :] + ar1t[:, :, None]        # [xd=32, h=8, t=32]
    ee = np.where(ee > 0, ee, 0.2 * ee)
    E_tab = np.exp(ee).astype(np.float32)             # [32, 8, 32]

    # cnt histogram over ALL edges incl self-loops
    src_all = np.concatenate([ei[0], np.arange(N)])
    dst_all = np.concatenate([ei[1], np.arange(N)])
    xs_all = x[src_all]
    cnt = np.zeros((N, TILE), np.float32)
    np.add.at(cnt, (dst_all, xs_all), 1.0)

    # ---- weight tables ----
    def wprime(W, a_s, a_d, H, C, pad_to):
        Fin = W.shape[0]
        As = np.zeros((H * C, H), np.float32)
        Ad = np.zeros((H * C, H), np.float32)
        for h in range(H):
            As[h * C:(h + 1) * C, h] = a_s[h]
            Ad[h * C:(h + 1) * C, h] = a_d[h]
        Wp = np.concatenate([W, W @ As, W @ Ad], axis=1)  # [Fin, H*C + 2H]
        out = np.zeros((Fin, pad_to), np.float32)
        out[:, :Wp.shape[1]] = Wp
        return out

    REC2 = 576   # 512 z + 8 al + 8 ar + 48 pad (f32)
    REC3 = 32    # 16 z + 1 al + 1 ar + 14 pad
    W2p = wprime(W2, as2, ad2, H2, C2, REC2)          # [512, 576]
    W3p = wprime(W3, as3, ad3, H3, C3, REC3)          # [512, 32]
    W2c = W2p.reshape(4, 128, REC2).copy()
    W3c = W3p.reshape(4, 128, REC3).copy()

    def bc(v, F):
        t = np.zeros((128, F), np.float32); t[:, :] = v[None, :F]; return t

    consts = dict(
        W2c=W2c, W3c=W3c,
        z1t=z1.astype(np.float32),                    # [32, 512]
        b1t=bc(b1, 512), g1t=bc(g1, 512), be1t=bc(be1, 512),
        b2t=bc(b2, 512), g2t=bc(g2, 512), be2t=bc(be2, 512),
        b3t=bc(b3, 16), g3t=bc(g3, 16), be3t=bc(be3, 16),
        iotaF=np.tile(np.arange(128, dtype=np.float32), (128, 1)),
        ident=np.eye(128, dtype=np.float32),
        onesc=np.ones((128, 1), np.float32),
    )

    # ---- per-core edge bucketing (non-self edges only) ----
    es, ed = ei[0], ei[1]
    core_of = ed // NPC
    grp_of = (ed % NPC) // 128
    # count per (core, group)
    counts = np.zeros((NC, NG), np.int64)
    np.add.at(counts, (core_of, grp_of), 1)
    Tg = np.maximum(1, ((counts.max(axis=0) + 127) // 128)).astype(np.int64)  # per-group tiles

    # gather index remap: node n -> row (n//NPC)*NPCP + n%NPC
    gidx_all = (es // NPC) * NPCP + (es % NPC)

    order = np.lexsort((es, grp_of, core_of))
    es_s, ed_s = es[order], ed[order]
    core_s, grp_s = core_of[order], grp_of[order]
    gidx_s = gidx_all[order]
    # boundaries per (core, group)
    starts = np.zeros((NC, NG), np.int64)
    flat = core_s * NG + grp_s
    bounds = np.searchsorted(flat, np.arange(NC * NG))
    starts = bounds.reshape(NC, NG)
    total = len(es_s)

    idx_src = np.zeros((NC, int(Tg.sum()) * 128), np.int32)
    dstloc = np.full((NC, int(Tg.sum()) * 128), 200.0, np.float32)
    toff = np.concatenate([[0], np.cumsum(Tg)]).astype(np.int64)  # tile offsets per group
    for c in range(NC):
        for g in range(NG):
            s = starts[c, g]
            e = starts[c, g + 1] if g + 1 < NG else (starts[c + 1, 0] if c + 1 < NC else total)
            n = e - s
            o = int(toff[g]) * 128
            cap = int(Tg[g]) * 128
            assert n <= cap, (c, g, n, cap)
            idx_src[c, o:o + n] = gidx_s[s:e]
            dstloc[c, o:o + n] = (ed_s[s:e] % NPC) % 128
    # reshape per group tile-major: slot j within group -> (tile j//128? ) We store
    # edge slot j at [tile=j//128 ... wait gather layout: out[p, t] = row idx[t*128+p]
    # => idx array per group laid out [T,128] with tile-major flattening, and the
    # SBUF idx tile loaded as [128, T] must be the transpose.
    NTT = int(Tg.sum())
    idx_src = idx_src.reshape(NC, NTT, 128)
    dstloc = dstloc.reshape(NC, NTT, 128)
    # SBUF-friendly layout [128, NTT]
    idx_srcT = np.ascontiguousarray(idx_src.transpose(0, 2, 1))   # [NC, 128, NTT]
    dstlocT = np.ascontiguousarray(dstloc.transpose(0, 2, 1))     # [NC, 128, NTT]

    # ---- per-core node arrays ----
    percore = []
    for c in range(NC):
        lo, hi = c * NPC, (c + 1) * NPC
        cntc = np.zeros((NPCP, TILE), np.float32)
        cntc[:NPC] = cnt[lo:hi]
        cntc[NPC:, 0] = 1.0  # pad rows: avoid 0/0
        Ec = np.zeros((NPCP, H1 * TILE), np.float32)
        Ec[:NPC] = E_tab[x[lo:hi]].reshape(NPC, H1 * TILE)
        Ec[NPC:] = 1.0
        batchc = np.full((NPCP, 1), 200.0, np.float32)
        batchc[:NPC, 0] = batch[lo:hi]
        percore.append(dict(
            cntc=cntc, Ec=Ec, batchc=batchc,
            idxs=idx_srcT[c], dls=dstlocT[c],
        ))

    meta = dict(N=N, E=E, G=G, NC=NC, NPC=NPC, NG=NG, NPCP=NPCP, Tg=Tg.tolist(),
                toff=toff.tolist(), REC2=REC2, REC3=REC3, H1=H1, C1=C1, H2=H2,
                C2=C2, H3=H3, C3=C3, TILE=TILE)
    host = dict(fcW1=np.asarray(inputs['fcW1'], np.float32),
                fcb1=np.asarray(inputs['fcb1'], np.float32),
                fcW2=np.asarray(inputs['fcW2'], np.float32),
                fcb2=np.asarray(inputs['fcb2'], np.float32),
                batch=batch)
    return consts, percore, meta, host


def layer_norm_elu(nc, pool, y, g_t, be_t, F, epsc=None):
    """In SBUF: y [128,F] -> elu(LN(y)*g+be). In-place heavy; returns new tile."""
    s1 = pool.tile([128, 1], F32, tag="ln_s1")
    nc.vector.tensor_reduce(out=s1[:], in_=y[:], axis=AX.X, op=ALU.add)
    m2 = pool.tile([128, 1], F32, tag="ln_m2")
    nc.vector.tensor_scalar_mul(out=m2[:], in0=s1[:], scalar1=-1.0 / F)
    sq = pool.tile([128, F], F32, tag="ln_sq")
    ss = pool.tile([128, 1], F32, tag="ln_ss")
    nc.scalar.activation(out=sq[:], in_=y[:], func=ACT.Square, bias=m2[:, :1],
                         accum_out=ss[:])
    sd = pool.tile([128, 1], F32, tag="ln_sd")
    nc.scalar.activation(out=sd[:], in_=ss[:], func=ACT.Sqrt, bias=epsc[:, :1], scale=1.0 / F)
    rs = pool.tile([128, 1], F32, tag="ln_rs")
    nc.vector.reciprocal(out=rs[:], in_=sd[:])
    # y <- (y - m) * istd ; then *g ; then +be   (in place)
    nc.vector.tensor_scalar(out=y[:], in0=y[:], scalar1=m2[:, :1], scalar2=rs[:, :1],
                            op0=ALU.add, op1=ALU.mult)
    nc.vector.tensor_tensor(out=y[:], in0=y[:], in1=g_t[:, :F], op=ALU.mult)
    nc.vector.tensor_tensor(out=y[:], in0=y[:], in1=be_t[:, :F], op=ALU.add)
    # ELU = max(x,0) + exp(min(x,0)) - 1 ; sq reused as scratch
    nc.vector.tensor_scalar_min(out=sq[:], in0=y[:], scalar1=0.0)
    nc.scalar.activation(out=sq[:], in_=sq[:], func=ACT.Exp)
    h = pool.tile([128, F], F32, tag="elu_h")
    nc.vector.tensor_scalar(out=h[:], in0=y[:], scalar1=0.0, scalar2=-1.0,
                            op0=ALU.max, op1=ALU.add)
    nc.vector.tensor_tensor(out=h[:], in0=h[:], in1=sq[:], op=ALU.add)
    return h


def transpose_128(nc, sb, pst, src_ap, ident, tag):
    """PE-transpose a [128,128] SBUF slice -> new SBUF tile."""
    pt = pst.tile([128, 128], F32, tag="tp_ps", space="PSUM")
    nc.tensor.transpose(out=pt[:], in_=src_ap, identity=ident[:])
    st = sb.tile([128, 128], F32, tag="tp_sb")
    nc.vector.tensor_copy(out=st[:], in_=pt[:])
    return st


def build(meta):
    NC, NG, NPCP = meta['NC'], meta['NG'], meta['NPCP']
    Tg, toff = meta['Tg'], meta['toff']
    NTT = toff[-1]
    REC2, REC3 = meta['REC2'], meta['REC3']
    G = meta['G']
    TILE, H1 = meta['TILE'], meta['H1']
    NFULL = NC * NPCP

    nc = bacc.Bacc("TRN2", num_devices=NC)
    # inputs
    t_cnt = nc.dram_tensor("cntc", [NPCP, TILE], F32, kind="ExternalInput")
    t_E = nc.dram_tensor("Ec", [NPCP, H1 * TILE], F32, kind="ExternalInput")
    t_bat = nc.dram_tensor("batchc", [NPCP, 1], F32, kind="ExternalInput")
    t_idx = nc.dram_tensor("idxs", [128, NTT], I32, kind="ExternalInput")
    t_dl = nc.dram_tensor("dls", [128, NTT], F32, kind="ExternalInput")
    t_W2c = nc.dram_tensor("W2c", [4, 128, REC2], F32, kind="ExternalInput")
    t_W3c = nc.dram_tensor("W3c", [4, 128, REC3], F32, kind="ExternalInput")
    t_z1t = nc.dram_tensor("z1t", [TILE, 512], F32, kind="ExternalInput")
    cn = {}
    for nm, sh in [("b1t", 512), ("g1t", 512), ("be1t", 512), ("b2t", 512),
                   ("g2t", 512), ("be2t", 512), ("b3t", 16), ("g3t", 16), ("be3t", 16)]:
        cn[nm] = nc.dram_tensor(nm, [128, sh], F32, kind="ExternalInput")
    t_iota = nc.dram_tensor("iotaF", [128, 128], F32, kind="ExternalInput")
    t_id = nc.dram_tensor("ident", [128, 128], F32, kind="ExternalInput")
    t_ones = nc.dram_tensor("onesc", [128, 1], F32, kind="ExternalInput")
    t_out = nc.dram_tensor("part", [G, 17], F32, kind="ExternalOutput")

    with tile.TileContext(nc) as tc:
        with tc.tile_pool(name="const", bufs=1) as cp, \
             tc.tile_pool(name="sb", bufs=2) as sb, \
             tc.tile_pool(name="gbuf", bufs=2) as gb, \
             tc.tile_pool(name="ps", bufs=1, space="PSUM") as ps, \
             tc.tile_pool(name="pst", bufs=2, space="PSUM") as pst, \
             tc.tile_pool(name="pacc", bufs=1, space="PSUM") as pacc, \
             tc.tile_pool(name="dram", bufs=1, space="DRAM") as dp:

            # ---- const loads ----
            C = {}
            for nm, src, shp in [("iotaF", t_iota, [128, 128]), ("ident", t_id, [128, 128]),
                                 ("z1t", t_z1t, [TILE, 512]), ("onesc", t_ones, [128, 1])]:
                C[nm] = cp.tile(shp, F32, tag="c_" + nm, name="c_" + nm)
                nc.sync.dma_start(out=C[nm][:], in_=src[:])
            for nm in cn:
                F = 512 if nm[-2] != '3' else 16
                C[nm] = cp.tile([128, F], F32, tag="c_" + nm, name="c_" + nm)
                nc.sync.dma_start(out=C[nm][:], in_=cn[nm][:])
            W2s = cp.tile([128, 4 * REC2], F32)
            nc.sync.dma_start(out=W2s[:].rearrange("p (a b) -> p a b", a=4), in_=t_W2c[:].rearrange("a p b -> p a b"))
            epsc = cp.tile([128, 1], F32, name="epsc")
            nc.vector.memset(epsc[:], EPS)
            W3s = cp.tile([128, 4 * REC3], F32)
            nc.sync.dma_start(out=W3s[:].rearrange("p (a b) -> p a b", a=4), in_=t_W3c[:].rearrange("a p b -> p a b"))

            rec2_sh = dp.tile([NPCP, REC2], F32)
            rec2_full = dp.tile([NFULL, REC2], F32, addr_space="Shared")
            rec3_sh = dp.tile([NPCP, REC3], F32)
            rec3_full = dp.tile([NFULL, REC3], F32, addr_space="Shared")

            # ================= L1 + phaseA(L2) =================
            for g in range(NG):
                r0 = g * 128
                cg = sb.tile([128, TILE], F32, tag="cg")
                nc.sync.dma_start(out=cg[:], in_=t_cnt[r0:r0 + 128, :])
                Eg = sb.tile([128, H1, TILE], F32, tag="Eg")
                nc.sync.dma_start(out=Eg[:, :, :], in_=t_E[r0:r0 + 128, :].rearrange("p (h t) -> p h t", h=H1))
                M = sb.tile([128, H1, TILE], F32, tag="M")
                nc.vector.tensor_tensor(out=M[:, :, :], in0=Eg[:, :, :],
                                        in1=cg[:, None, :].to_broadcast([128, H1, TILE]),
                                        op=ALU.mult)
                s = sb.tile([128, H1], F32, tag="s")
                nc.vector.tensor_reduce(out=s[:], in_=M[:, :, :], axis=AX.X, op=ALU.add)
                rs = sb.tile([128, H1], F32, tag="rs")
                nc.vector.reciprocal(out=rs[:], in_=s[:])
                nc.vector.tensor_tensor(out=M[:, :, :], in0=M[:, :, :],
                                        in1=rs[:, :, None].to_broadcast([128, H1, TILE]),
                                        op=ALU.mult)
                P = M
                pO = ps.tile([128, 512], F32, tag="pacc_main", space="PSUM")
                for h in range(H1):
                    ptp = pst.tile([128, 128], F32, tag="tp_ps", space="PSUM")
                    nc.tensor.transpose(out=ptp[:TILE, :], in_=P[:, h, :], identity=C["ident"][:])
                    PT = sb.tile([TILE, 128], F32, tag="PT")
                    nc.vector.tensor_copy(out=PT[:], in_=ptp[:TILE, :])
                    nc.tensor.matmul(out=pO[:, h * 64:(h + 1) * 64], lhsT=PT[:],
                                     rhs=C["z1t"][:, h * 64:(h + 1) * 64],
                                     start=True, stop=True)
                y = sb.tile([128, 512], F32, tag="y1")
                nc.vector.tensor_tensor(out=y[:], in0=pO[:], in1=C["b1t"][:], op=ALU.add)
                h1 = layer_norm_elu(nc, sb, y, C["g1t"], C["be1t"], 512, epsc)
                # transpose h1 -> 4 chunks, phase-A W2'
                z2p = ps.tile([128, 512], F32, tag="pz", space="PSUM")
                z2pb = ps.tile([128, 64], F32, tag="z2pb", space="PSUM")
                for k in range(4):
                    hT = transpose_128(nc, sb, pst, h1[:, k * 128:(k + 1) * 128], C["ident"], "h1T")
                    nc.tensor.matmul(out=z2p[:], lhsT=hT[:], rhs=W2s[:, k * REC2:k * REC2 + 512],
                                     start=(k == 0), stop=(k == 3))
                    nc.tensor.matmul(out=z2pb[:], lhsT=hT[:], rhs=W2s[:, k * REC2 + 512:(k + 1) * REC2],
                                     start=(k == 0), stop=(k == 3))
                zs = sb.tile([128, REC2], F32, tag="zs")
                nc.vector.tensor_copy(out=zs[:, :512], in_=z2p[:])
                nc.vector.tensor_copy(out=zs[:, 512:], in_=z2pb[:])
                nc.sync.dma_start(out=rec2_sh[r0:r0 + 128, :], in_=zs[:])

            nc.gpsimd.collective_compute(
                "AllGather", ALU.bypass, replica_groups=[list(range(NC))],
                ins=[rec2_sh.opt()], outs=[rec2_full.opt()])

            # ================= L2 + phaseA(L3) =================
            for g in range(NG):
                r0 = g * 128
                T = Tg[g]
                o0 = toff[g]
                ig = sb.tile([128, T], I32, tag="ig")
                nc.sync.dma_start(out=ig[:], in_=t_idx[:, o0:o0 + T])
                dl = sb.tile([128, T], F32, tag="dl")
                nc.sync.dma_start(out=dl[:], in_=t_dl[:, o0:o0 + T])
                zg = sb.tile([128, REC2], F32, tag="zg")
                nc.sync.dma_start(out=zg[:], in_=rec2_sh[r0:r0 + 128, :])
                Gt = gb.tile([128, T, REC2], F32, tag="G")
                for t in range(T):
                    nc.gpsimd.indirect_dma_start(
                        out=Gt[:, t, :], out_offset=None, in_=rec2_full[:],
                        in_offset=bass.IndirectOffsetOnAxis(ap=ig[:, t:t + 1], axis=0))
                S = gb.tile([128, T, 128], F32, tag="S")
                nc.vector.tensor_tensor(
                    out=S[:, :, :],
                    in0=C["iotaF"][:, None, :].to_broadcast([128, T, 128]),
                    in1=dl[:, :, None].to_broadcast([128, T, 128]),
                    op=ALU.is_equal)
                H2x, C2x = 4, 128
                pAR = ps.tile([128, T * H2x], F32, tag="pAR", space="PSUM")
                for t in range(T):
                    STt = transpose_128(nc, sb, pst, S[:, t, :], C["ident"], "ST")
                    nc.tensor.matmul(out=pAR[:, t * H2x:(t + 1) * H2x], lhsT=STt[:],
                                     rhs=zg[:, 516:520], start=True, stop=True)
                eL = sb.tile([128, T * H2x], F32, tag="eL")
                nc.vector.tensor_tensor(
                    out=eL[:].rearrange("p (t h) -> p t h", h=H2x),
                    in0=Gt[:, :, 512:516], in1=pAR[:].rearrange("p (t h) -> p t h", h=H2x),
                    op=ALU.add)
                eA = sb.tile([128, T * H2x], F32, tag="eA")
                nc.vector.tensor_scalar_mul(out=eA[:], in0=eL[:], scalar1=0.2)
                nc.vector.tensor_tensor(out=eA[:], in0=eL[:], in1=eA[:], op=ALU.max)
                EX = sb.tile([128, T * H2x], F32, tag="EX")
                nc.scalar.activation(out=EX[:], in_=eA[:], func=ACT.Exp)
                # scale z-part of G by EX (per head block of C2x)
                nc.vector.tensor_tensor(
                    out=Gt[:, :, :512].rearrange("p t (h c) -> p t h c", h=H2x),
                    in0=Gt[:, :, :512].rearrange("p t (h c) -> p t h c", h=H2x),
                    in1=EX[:].rearrange("p (t h) -> p t h", h=H2x)[:, :, :, None]
                        .to_broadcast([128, T, H2x, C2x]),
                    op=ALU.mult)
                pMain = ps.tile([128, 512], F32, tag="pacc_main", space="PSUM")
                pS = ps.tile([128, H2x], F32, tag="pacc_s", space="PSUM")
                for t in range(T):
                    nc.tensor.matmul(out=pMain[:], lhsT=S[:, t, :], rhs=Gt[:, t, :512],
                                     start=(t == 0), stop=(t == T - 1))
                    nc.tensor.matmul(out=pS[:], lhsT=S[:, t, :], rhs=EX[:, t * H2x:(t + 1) * H2x],
                                     start=(t == 0), stop=(t == T - 1))
                # self-loop
                eSl = sb.tile([128, H2x], F32, tag="eSl")
                nc.vector.tensor_tensor(out=eSl[:], in0=zg[:, 512:516], in1=zg[:, 516:520], op=ALU.add)
                eSa = sb.tile([128, H2x], F32, tag="eSa")
                nc.vector.tensor_scalar_mul(out=eSa[:], in0=eSl[:], scalar1=0.2)
                nc.vector.tensor_tensor(out=eSa[:], in0=eSl[:], in1=eSa[:], op=ALU.max)
                exS = sb.tile([128, H2x], F32, tag="exS")
                nc.scalar.activation(out=exS[:], in_=eSa[:], func=ACT.Exp)
                selfc = sb.tile([128, 512], F32, tag="selfc")
                nc.vector.tensor_tensor(
                    out=selfc[:].rearrange("p (h c) -> p h c", h=H2x),
                    in0=zg[:, :512].rearrange("p (h c) -> p h c", h=H2x),
                    in1=exS[:, :, None].to_broadcast([128, H2x, C2x]), op=ALU.mult)
                nc.vector.tensor_tensor(out=selfc[:], in0=pMain[:], in1=selfc[:], op=ALU.add)
                sS = sb.tile([128, H2x], F32, tag="sS")
                nc.vector.tensor_tensor(out=sS[:], in0=pS[:], in1=exS[:], op=ALU.add)
                rS = sb.tile([128, H2x], F32, tag="rS")
                nc.vector.reciprocal(out=rS[:], in_=sS[:])
                nc.vector.tensor_tensor(
                    out=selfc[:].rearrange("p (h c) -> p h c", h=H2x),
                    in0=selfc[:].rearrange("p (h c) -> p h c", h=H2x),
                    in1=rS[:, :, None].to_broadcast([128, H2x, C2x]), op=ALU.mult)
                nc.vector.tensor_tensor(out=selfc[:], in0=selfc[:], in1=C["b2t"][:], op=ALU.add)
                h2 = layer_norm_elu(nc, sb, selfc, C["g2t"], C["be2t"], 512, epsc)
                z3p = ps.tile([128, REC3], F32, tag="pz", space="PSUM")
                for k in range(4):
                    hT = transpose_128(nc, sb, pst, h2[:, k * 128:(k + 1) * 128], C["ident"], "h2T")
                    nc.tensor.matmul(out=z3p[:], lhsT=hT[:], rhs=W3s[:, k * REC3:(k + 1) * REC3],
                                     start=(k == 0), stop=(k == 3))
                z3s = sb.tile([128, REC3], F32, tag="z3s")
                nc.vector.tensor_copy(out=z3s[:], in_=z3p[:])
                nc.sync.dma_start(out=rec3_sh[r0:r0 + 128, :], in_=z3s[:])

            nc.gpsimd.collective_compute(
                "AllGather", ALU.bypass, replica_groups=[list(range(NC))],
                ins=[rec3_sh.opt()], outs=[rec3_full.opt()])

            # ================= L3 + pooling =================
            pPool = pacc.tile([128, 17], F32, tag="pPool", space="PSUM")
            for g in range(NG):
                r0 = g * 128
                T = Tg[g]
                o0 = toff[g]
                ig = sb.tile([128, T], I32, tag="ig3")
                nc.sync.dma_start(out=ig[:], in_=t_idx[:, o0:o0 + T])
                dl = sb.tile([128, T], F32, tag="dl3")
                nc.sync.dma_start(out=dl[:], in_=t_dl[:, o0:o0 + T])
                zg = sb.tile([128, REC3], F32, tag="zg3")
                nc.sync.dma_start(out=zg[:], in_=rec3_sh[r0:r0 + 128, :])
                bg = sb.tile([128, 1], F32, tag="bg")
                nc.sync.dma_start(out=bg[:], in_=t_bat[r0:r0 + 128, :])
                Gt = gb.tile([128, T, REC3], F32, tag="G")
                for t in range(T):
                    nc.gpsimd.indirect_dma_start(
                        out=Gt[:, t, :], out_offset=None, in_=rec3_full[:],
                        in_offset=bass.IndirectOffsetOnAxis(ap=ig[:, t:t + 1], axis=0))
                S = gb.tile([128, T, 128], F32, tag="S")
                nc.vector.tensor_tensor(
                    out=S[:, :, :],
                    in0=C["iotaF"][:, None, :].to_broadcast([128, T, 128]),
                    in1=dl[:, :, None].to_broadcast([128, T, 128]),
                    op=ALU.is_equal)
                pAR = ps.tile([128, T], F32, tag="pAR", space="PSUM")
                for t in range(T):
                    STt = transpose_128(nc, sb, pst, S[:, t, :], C["ident"], "ST3")
                    nc.tensor.matmul(out=pAR[:, t:t + 1], lhsT=STt[:],
                                     rhs=zg[:, 17:18], start=True, stop=True)
                eL = sb.tile([128, T], F32, tag="eL3")
                nc.vector.tensor_tensor(out=eL[:], in0=Gt[:, :, 16], in1=pAR[:], op=ALU.add)
                eA = sb.tile([128, T], F32, tag="eA3")
                nc.vector.tensor_scalar_mul(out=eA[:], in0=eL[:], scalar1=0.2)
                nc.vector.tensor_tensor(out=eA[:], in0=eL[:], in1=eA[:], op=ALU.max)
                EX = sb.tile([128, T], F32, tag="EX3")
                nc.scalar.activation(out=EX[:], in_=eA[:], func=ACT.Exp)
                nc.vector.tensor_tensor(
                    out=Gt[:, :, :16], in0=Gt[:, :, :16],
                    in1=EX[:, :, None].to_broadcast([128, T, 16]), op=ALU.mult)
                pM3 = ps.tile([128, 16], F32, tag="pacc_main", space="PSUM")
                pS3 = ps.tile([128, 1], F32, tag="pacc_s", space="PSUM")
                for t in range(T):
                    nc.tensor.matmul(out=pM3[:], lhsT=S[:, t, :], rhs=Gt[:, t, :16],
                                     start=(t == 0), stop=(t == T - 1))
                    nc.tensor.matmul(out=pS3[:], lhsT=S[:, t, :], rhs=EX[:, t:t + 1],
                                     start=(t == 0), stop=(t == T - 1))
                eSl = sb.tile([128, 1], F32, tag="eSl3")
                nc.vector.tensor_tensor(out=eSl[:], in0=zg[:, 16:17], in1=zg[:, 17:18], op=ALU.add)
                eSa = sb.tile([128, 1], F32, tag="eSa3")
                nc.vector.tensor_scalar_mul(out=eSa[:], in0=eSl[:], scalar1=0.2)
                nc.vector.tensor_tensor(out=eSa[:], in0=eSl[:], in1=eSa[:], op=ALU.max)
                exS = sb.tile([128, 1], F32, tag="exS3")
                nc.scalar.activation(out=exS[:], in_=eSa[:], func=ACT.Exp)
                selfc = sb.tile([128, 16], F32, tag="selfc3")
                nc.vector.tensor_scalar(out=selfc[:], in0=zg[:, :16], scalar1=exS[:, :1],
                                        scalar2=None, op0=ALU.mult)
                nc.vector.tensor_tensor(out=selfc[:], in0=pM3[:], in1=selfc[:], op=ALU.add)
                sS = sb.tile([128, 1], F32, tag="sS3")
                nc.vector.tensor_tensor(out=sS[:], in0=pS3[:], in1=exS[:], op=ALU.add)
                rS = sb.tile([128, 1], F32, tag="rS3")
                nc.vector.reciprocal(out=rS[:], in_=sS[:])
                nc.vector.tensor_scalar(out=selfc[:], in0=selfc[:], scalar1=rS[:, :1],
                                        scalar2=None, op0=ALU.mult)
                nc.vector.tensor_tensor(out=selfc[:], in0=selfc[:], in1=C["b3t"][:], op=ALU.add)
                h3 = layer_norm_elu(nc, sb, selfc, C["g3t"], C["be3t"], 16, epsc)
                OB = sb.tile([128, G], F32, tag="OB")
                nc.vector.tensor_tensor(
                    out=OB[:], in0=C["iotaF"][:, :G],
                    in1=bg[:, :1].to_broadcast([128, G]), op=ALU.is_equal)
                h3w = sb.tile([128, 17], F32, tag="h3w")
                nc.vector.tensor_copy(out=h3w[:, :16], in_=h3[:])
                nc.vector.memset(h3w[:, 16:17], 1.0)
                nc.tensor.matmul(out=pPool[:G, :17], lhsT=OB[:], rhs=h3w[:],
                                 start=(g == 0), stop=(g == NG - 1))
            po = sb.tile([128, 17], F32, tag="po")
            nc.vector.tensor_copy(out=po[:G, :], in_=pPool[:G, :])
            nc.sync.dma_start(out=t_out[:, :], in_=po[:G, :])
    nc.finalize()
    return nc


def run(inputs, N, E, G, NC, runner, TILE=32, EMB=16):
    consts, percore, meta, host = host_prep(inputs, N, E, G, NC, TILE, EMB)
    nc = build(meta)
    in_maps = []
    for c in range(NC):
        m = dict(consts)
        m.update(percore[c])
        in_maps.append(m)
    results = runner(nc, in_maps)
    parts = np.stack([r["part"] for r in results])  # [NC, G, 17]
    tot = parts.sum(axis=0)
    pooled = tot[:, :16] / np.maximum(tot[:, 16:17], 1.0)
    h = np.maximum(pooled @ host['fcW1'] + host['fcb1'], 0.0)
    return (h @ host['fcW2'] + host['fcb2']).astype(np.float32)


# ======================= kernel entry =======================
N_FULL, E_FULL, G_FULL, NC_FULL = 50000, 800000, 50, 8
_CACHE = {}


def _hw_runner(nc, in_maps):
    from concourse.bass_utils import run_bass_kernel_spmd
    res = run_bass_kernel_spmd(nc, in_maps, core_ids=list(range(len(in_maps))))
    return res.results


def kernel(**inputs):
    consts, percore, meta, host = host_prep(inputs, N_FULL, E_FULL, G_FULL, NC_FULL)
    key = tuple(meta['Tg'])
    if key not in _CACHE:
        _CACHE[key] = build(meta)
    nc = _CACHE[key]
    in_maps = []
    for c in range(NC_FULL):
        m = dict(consts)
        m.update(percore[c])
        in_maps.append(m)
    results = _hw_runner(nc, in_maps)
    parts = np.stack([r["part"] for r in results])
    tot = parts.sum(axis=0)
    pooled = tot[:, :16] / np.maximum(tot[:, 16:17], 1.0)
    h = np.maximum(pooled @ host['fcW1'] + host['fcb1'], 0.0)
    return (h @ host['fcW2'] + host['fcb2']).astype(np.float32)



# revision 8
# speedup vs baseline: 1.2397x; 1.2397x over previous
"""MinamoTopoModel GAT kernel: host preprocessing + Bass builder.

8-core SPMD, dst-sharded. v2 design:
  L1: cnt-histogram trick -> blockdiag matmuls (2 per group), no edge work.
  L2/L3: per-group batched dma_gather of src node records (bf16, one
         instruction per group-half instead of one indirect DMA per 128
         edges), ar[dst] via a second local dma_gather, S-matrix
         (iota-compare) PSUM scatter matmuls in bf16, segment softmax
         without max-subtraction, self-loops handled per-group directly.
  Chunked AllGathers (7 x 896 rows) publish per-shard node records between
  layers, overlapping the collective with the producing loop.
  Graph pooling -> per-core [50,17] partials; final FC on host.

rec2_full rows exceed int16 gather-index range (50176 > 32767), so edges
are split per group into two gathers against the low/high half-tables.
"""
import numpy as np
import ml_dtypes
import concourse.bacc as bacc
import concourse.bass as bass
import concourse.mybir as mybir
import concourse.tile as tile

F32 = mybir.dt.float32
BF16 = mybir.dt.bfloat16
I16 = mybir.dt.int16
AX = mybir.AxisListType
ALU = mybir.AluOpType
ACT = mybir.ActivationFunctionType
EPS = 1e-5
BF = ml_dtypes.bfloat16

N_FULL, E_FULL, G_FULL, NC_FULL = 50000, 800000, 50, 8
TILE, EMB = 32, 16
H1, C1, H2, C2, H3, C3 = 8, 64, 4, 128, 1, 16
REC2 = 640    # bf16: 512 z + 4 al + 4 ar + 120 pad  (1280B, %256==0)
REC3 = 128    # bf16: 16 z + 1 al + 1 ar + 110 pad   (256B)
NCHUNK = 1    # AllGather chunks (tile framework: single writer per Shared tensor)


def _wrap16(arr):
    """int16 idx list (len%16==0) -> [128, n/16] wrapped + replicated."""
    n = len(arr)
    w = arr.reshape(n // 16, 16).T            # [16, n/16]
    return np.tile(w, (8, 1)).astype(np.int16)


def host_prep(inputs, N, E, G, NC):
    x = np.asarray(inputs['x']).astype(np.int64)
    ei = np.asarray(inputs['edge_index']).astype(np.int64)
    batch = np.asarray(inputs['batch']).astype(np.int64)
    emb = np.asarray(inputs['emb'], np.float32)
    W1 = np.asarray(inputs['W1'], np.float32)
    as1 = np.asarray(inputs['a_src1'], np.float32); ad1 = np.asarray(inputs['a_dst1'], np.float32)
    b1 = np.asarray(inputs['b1'], np.float32)
    g1 = np.asarray(inputs['g1'], np.float32); be1 = np.asarray(inputs['be1'], np.float32)
    W2 = np.asarray(inputs['W2'], np.float32)
    as2 = np.asarray(inputs['a_src2'], np.float32); ad2 = np.asarray(inputs['a_dst2'], np.float32)
    b2 = np.asarray(inputs['b2'], np.float32)
    g2 = np.asarray(inputs['g2'], np.float32); be2 = np.asarray(inputs['be2'], np.float32)
    W3 = np.asarray(inputs['W3'], np.float32)
    as3 = np.asarray(inputs['a_src3'], np.float32); ad3 = np.asarray(inputs['a_dst3'], np.float32)
    b3 = np.asarray(inputs['b3'], np.float32)
    g3 = np.asarray(inputs['g3'], np.float32); be3 = np.asarray(inputs['be3'], np.float32)

    NPC = N // NC                      # 6250
    NG = (NPC + 127) // 128            # 49
    NPCP = NG * 128                    # 6272
    CH = NPCP // NCHUNK                # 896 rows per AllGather chunk
    CHF = CH * NC                      # 7168 rows per full chunk
    NFULL = NC * NPCP                  # 50176
    HALF = NFULL // 2                  # 25088

    # ---- L1 tables (cnt trick) ----
    z1 = emb @ W1                                     # [32, 512]
    z1h = z1.reshape(TILE, H1, C1)
    al1t = np.einsum('thc,hc->th', z1h, as1)          # [32,8]
    ar1t = np.einsum('thc,hc->th', z1h, ad1)
    ee = al1t.T[None, :, :] + ar1t[:, :, None]        # [xd=32, h=8, t=32]
    ee = np.where(ee > 0, ee, 0.2 * ee)
    E_tab = np.exp(ee).astype(np.float32)             # [32, 8, 32]

    src_all = np.concatenate([ei[0], np.arange(N)])
    dst_all = np.concatenate([ei[1], np.arange(N)])
    xs_all = x[src_all]
    cnt = np.zeros((N, TILE), np.float32)
    np.add.at(cnt, (dst_all, xs_all), 1.0)

    # blockdiag Z1B: Z1B[hb][h*32+t, h*64+c] = z1[t, (hb*4+h)*64+c]
    Z1B = np.zeros((2, 128, 256), np.float32)
    for hb in range(2):
        for h in range(4):
            Z1B[hb, h * 32:(h + 1) * 32, h * 64:(h + 1) * 64] = \
                z1[:, (hb * 4 + h) * 64:(hb * 4 + h + 1) * 64]

    # ---- weight tables: W' = [W | W@As | W@Ad] ----
    def wprime(W, a_s, a_d, H, C):
        As = np.zeros((H * C, H), np.float32)
        Ad = np.zeros((H * C, H), np.float32)
        for h in range(H):
            As[h * C:(h + 1) * C, h] = a_s[h]
            Ad[h * C:(h + 1) * C, h] = a_d[h]
        return np.concatenate([W, W @ As, W @ Ad], axis=1)  # [512, H*C+2H]

    W2p = wprime(W2, as2, ad2, H2, C2)   # [512, 520]
    W3p = wprime(W3, as3, ad3, H3, C3)   # [512, 18]
    W2s = np.concatenate([W2p[k * 128:(k + 1) * 128] for k in range(4)], axis=1)  # [128, 4*520]
    W3s = np.concatenate([W3p[k * 128:(k + 1) * 128] for k in range(4)], axis=1)  # [128, 4*18]

    def bc(v, F):
        t = np.zeros((128, F), np.float32); t[:, :] = v[None, :F]; return t

    iota = np.tile(np.arange(128, dtype=np.float32), (128, 1))
    consts = dict(
        Z1B0=Z1B[0].astype(BF), Z1B1=Z1B[1].astype(BF),
        W2s=W2s.astype(BF), W3s=W3s.astype(BF),
        b1t=bc(b1, 512), g1t=bc(g1, 512), be1t=bc(be1, 512),
        b2t=bc(b2, 512), g2t=bc(g2, 512), be2t=bc(be2, 512),
        b3t=bc(b3, 16), g3t=bc(g3, 16), be3t=bc(be3, 16),
        iotab=iota.astype(BF),
        identb=np.eye(128, dtype=BF),
    )

    # ---- per-core edge bucketing ----
    es, ed = ei[0], ei[1]
    # global gathered-row of a node (chunked AllGather layout)
    c0 = es // NPC
    r = es % NPC
    kk = r // CH
    grow = kk * CHF + c0 * CH + (r - kk * CH)       # [E]
    half = (grow >= HALF).astype(np.int64)

    core_of = ed // NPC
    rd = ed % NPC
    grp_of = rd // 128
    dstloc = rd % 128

    # counts per (core, group, half)
    cnts = np.zeros((NC, NG, 2), np.int64)
    np.add.at(cnts, (core_of, grp_of, half), 1)
    TA = np.maximum(1, (cnts[:, :, 0].max(axis=0) + 127) // 128)  # [NG]
    TB = np.maximum(1, (cnts[:, :, 1].max(axis=0) + 127) // 128)
    Tg = TA + TB
    toff = np.concatenate([[0], np.cumsum(Tg)]).astype(np.int64)
    NTT = int(toff[-1])

    order = np.lexsort((es, half, grp_of, core_of))
    es_s = es[order]; half_s = half[order]
    core_s, grp_s = core_of[order], grp_of[order]
    grow_s = grow[order]; dstloc_s = dstloc[order]; rd_s = rd[order]
    flat = (core_s * NG + grp_s) * 2 + half_s
    bounds = np.searchsorted(flat, np.arange(NC * NG * 2 + 1))

    percore = []
    for c in range(NC):
        idxg = np.zeros((NTT * 128,), np.int64)
        idxd = np.zeros((NTT * 128,), np.int64)
        dlf = np.full((NTT * 128,), 200.0, np.float32)
        for g in range(NG):
            o = int(toff[g]) * 128
            for h in range(2):
                f = (c * NG + g) * 2 + h
                s, e = bounds[f], bounds[f + 1]
                n = e - s
                cap = int((TA if h == 0 else TB)[g]) * 128
                assert n <= cap, (c, g, h, n, cap)
                oo = o + (int(TA[g]) * 128 if h else 0)
                idxg[oo:oo + n] = grow_s[s:e] - HALF * h
                idxd[oo:oo + n] = rd_s[s:e]
                dlf[oo:oo + n] = dstloc_s[s:e]
        idxg16 = _wrap16(idxg.astype(np.int16))          # [128, NTT*8]
        idxd16 = _wrap16(idxd.astype(np.int16))
        dls = np.ascontiguousarray(
            dlf.reshape(NTT, 128).T).astype(BF)          # [128, NTT]

        lo, hi = c * NPC, (c + 1) * NPC
        cntc = np.zeros((NPCP, TILE), np.float32)
        cntc[:NPC] = cnt[lo:hi]
        cntc[NPC:, 0] = 1.0
        Ec = np.zeros((NPCP, H1 * TILE), np.float32)
        Ec[:NPC] = E_tab[x[lo:hi]].reshape(NPC, H1 * TILE)
        Ec[NPC:] = 1.0
        batchc = np.full((NPCP, 1), 200.0, np.float32)
        batchc[:NPC, 0] = batch[lo:hi]
        percore.append(dict(
            cntc=cntc, Ec=Ec, batchc=batchc.astype(BF),
            idxg16=idxg16, idxd16=idxd16, dls=dls,
        ))

    meta = dict(N=N, E=E, G=G, NC=NC, NPC=NPC, NG=NG, NPCP=NPCP,
                CH=CH, CHF=CHF, NFULL=NFULL, HALF=HALF,
                TA=TA.tolist(), TB=TB.tolist(), Tg=Tg.tolist(),
                toff=toff.tolist())
    host = dict(fcW1=np.asarray(inputs['fcW1'], np.float32),
                fcb1=np.asarray(inputs['fcb1'], np.float32),
                fcW2=np.asarray(inputs['fcW2'], np.float32),
                fcb2=np.asarray(inputs['fcb2'], np.float32))
    return consts, percore, meta, host


def emit_gather(nc, out3, table, idxt, ntiles, elem, elem_step=None, t0=0, i0=0):
    """dma_gather capped at 1024 idxs (8 tiles) per instruction.

    out3: [128, T, elem] SBUF tile (writes tiles [t0, t0+ntiles));
    idxt: [128, ncols] int16 SBUF tile (reads cols [i0*8, (i0+ntiles)*8)).
    """
    CAP = 8
    t = 0
    while t < ntiles:
        n = min(CAP, ntiles - t)
        nc.gpsimd.dma_gather(
            out3[:, t0 + t:t0 + t + n, :], table,
            idxt[:, (i0 + t) * 8:(i0 + t + n) * 8],
            n * 128, n * 128, elem, elem_step=elem_step)
        t += n


def layer_norm_elu(nc, pool, y, g_t, be_t, F, epsc):
    """In SBUF f32: y [128,F] -> elu(LN(y)*g+be). Returns new tile."""
    s1 = pool.tile([128, 1], F32, tag="ln_s1")
    nc.vector.tensor_reduce(out=s1[:], in_=y[:], axis=AX.X, op=ALU.add)
    m2 = pool.tile([128, 1], F32, tag="ln_m2")
    nc.vector.tensor_scalar_mul(out=m2[:], in0=s1[:], scalar1=-1.0 / F)
    sq = pool.tile([128, F], F32, tag="ln_sq")
    ss = pool.tile([128, 1], F32, tag="ln_ss")
    nc.scalar.activation(out=sq[:], in_=y[:], func=ACT.Square, bias=m2[:, :1],
                         accum_out=ss[:])
    sd = pool.tile([128, 1], F32, tag="ln_sd")
    nc.scalar.activation(out=sd[:], in_=ss[:], func=ACT.Sqrt, bias=epsc[:, :1], scale=1.0 / F)
    rs = pool.tile([128, 1], F32, tag="ln_rs")
    nc.vector.reciprocal(out=rs[:], in_=sd[:])
    nc.vector.tensor_scalar(out=y[:], in0=y[:], scalar1=m2[:, :1], scalar2=rs[:, :1],
                            op0=ALU.add, op1=ALU.mult)
    nc.vector.tensor_tensor(out=y[:], in0=y[:], in1=g_t[:, :F], op=ALU.mult)
    nc.vector.tensor_tensor(out=y[:], in0=y[:], in1=be_t[:, :F], op=ALU.add)
    nc.vector.tensor_scalar_min(out=sq[:], in0=y[:], scalar1=0.0)
    nc.scalar.activation(out=sq[:], in_=sq[:], func=ACT.Exp)
    h = pool.tile([128, F], F32, tag="elu_h")
    nc.vector.tensor_scalar(out=h[:], in0=y[:], scalar1=0.0, scalar2=-1.0,
                            op0=ALU.max, op1=ALU.add)
    nc.vector.tensor_tensor(out=h[:], in0=h[:], in1=sq[:], op=ALU.add)
    return h


def build(meta):
    NC, NG, NPCP = meta['NC'], meta['NG'], meta['NPCP']
    TA, TB, Tg, toff = meta['TA'], meta['TB'], meta['Tg'], meta['toff']
    NTT = toff[-1]
    CH, NFULL, HALF = meta['CH'], meta['NFULL'], meta['HALF']
    CHF = meta['CHF']
    G = meta['G']

    nc = bacc.Bacc("TRN2", num_devices=NC)
    t_cnt = nc.dram_tensor("cntc", [NPCP, TILE], F32, kind="ExternalInput")
    t_E = nc.dram_tensor("Ec", [NPCP, H1 * TILE], F32, kind="ExternalInput")
    t_bat = nc.dram_tensor("batchc", [NPCP, 1], BF16, kind="ExternalInput")
    t_ig = nc.dram_tensor("idxg16", [128, NTT * 8], I16, kind="ExternalInput")
    t_id = nc.dram_tensor("idxd16", [128, NTT * 8], I16, kind="ExternalInput")
    t_dl = nc.dram_tensor("dls", [128, NTT], BF16, kind="ExternalInput")
    t_Z0 = nc.dram_tensor("Z1B0", [128, 256], BF16, kind="ExternalInput")
    t_Z1 = nc.dram_tensor("Z1B1", [128, 256], BF16, kind="ExternalInput")
    t_W2s = nc.dram_tensor("W2s", [128, 4 * 520], BF16, kind="ExternalInput")
    t_W3s = nc.dram_tensor("W3s", [128, 4 * 18], BF16, kind="ExternalInput")
    cn = {}
    for nm, sh in [("b1t", 512), ("g1t", 512), ("be1t", 512), ("b2t", 512),
                   ("g2t", 512), ("be2t", 512), ("b3t", 16), ("g3t", 16), ("be3t", 16)]:
        cn[nm] = nc.dram_tensor(nm, [128, sh], F32, kind="ExternalInput")
    t_iob = nc.dram_tensor("iotab", [128, 128], BF16, kind="ExternalInput")
    t_idb = nc.dram_tensor("identb", [128, 128], BF16, kind="ExternalInput")
    t_out = nc.dram_tensor("part", [G, 17], F32, kind="ExternalOutput")

    with tile.TileContext(nc) as tc:
        with tc.tile_pool(name="const", bufs=1) as cp, \
             tc.tile_pool(name="sb", bufs=2) as sb, \
             tc.tile_pool(name="gbuf", bufs=2) as gb, \
             tc.tile_pool(name="ps", bufs=1, space="PSUM") as ps, \
             tc.tile_pool(name="pst", bufs=2, space="PSUM") as pst, \
             tc.tile_pool(name="pacc", bufs=1, space="PSUM") as pacc, \
             tc.tile_pool(name="dram", bufs=1, space="DRAM") as dp:

            # ---- const loads ----
            C = {}
            for nm, src, shp, dt in [
                    ("iotab", t_iob, [128, 128], BF16),
                    ("identb", t_idb, [128, 128], BF16),
                    ("Z1B0", t_Z0, [128, 256], BF16),
                    ("Z1B1", t_Z1, [128, 256], BF16),
                    ("W2s", t_W2s, [128, 4 * 520], BF16),
                    ("W3s", t_W3s, [128, 4 * 18], BF16)]:
                C[nm] = cp.tile(shp, dt, tag="c_" + nm, name="c_" + nm)
                nc.sync.dma_start(out=C[nm][:], in_=src[:])
            for nm in cn:
                F = 512 if nm[-2] != '3' else 16
                C[nm] = cp.tile([128, F], F32, tag="c_" + nm, name="c_" + nm)
                nc.sync.dma_start(out=C[nm][:], in_=cn[nm][:])
            epsc = cp.tile([128, 1], F32, name="epsc")
            nc.vector.memset(epsc[:], EPS)

            rec2_sh = dp.tile([NPCP, REC2], BF16)
            rec2_full = dp.tile([NFULL, REC2], BF16, addr_space="Shared")
            rec3_sh = dp.tile([NPCP, REC3], BF16)
            rec3_full = dp.tile([NFULL, REC3], BF16, addr_space="Shared")

            GPC = NG // NCHUNK  # groups per AllGather chunk (7)

            # ================= L1 + phaseA(L2) =================
            for g in range(NG):
                r0 = g * 128
                cg = sb.tile([128, TILE], F32, tag="cg")
                nc.sync.dma_start(out=cg[:], in_=t_cnt[r0:r0 + 128, :])
                Eg = sb.tile([128, H1, TILE], F32, tag="Eg")
                nc.sync.dma_start(out=Eg[:, :, :], in_=t_E[r0:r0 + 128, :].rearrange("p (h t) -> p h t", h=H1))
                M = sb.tile([128, H1, TILE], F32, tag="M")
                nc.vector.tensor_tensor(out=M[:, :, :], in0=Eg[:, :, :],
                                        in1=cg[:, None, :].to_broadcast([128, H1, TILE]),
                                        op=ALU.mult)
                s = sb.tile([128, H1], F32, tag="s")
                nc.vector.tensor_reduce(out=s[:], in_=M[:, :, :], axis=AX.X, op=ALU.add)
                rs = sb.tile([128, H1], F32, tag="rs")
                nc.vector.reciprocal(out=rs[:], in_=s[:])
                nc.vector.tensor_tensor(out=M[:, :, :], in0=M[:, :, :],
                                        in1=rs[:, :, None].to_broadcast([128, H1, TILE]),
                                        op=ALU.mult)
                Pb = sb.tile([128, 256], BF16, tag="Pb")
                nc.vector.tensor_copy(out=Pb[:].rearrange("p (h t) -> p h t", h=H1),
                                      in_=M[:, :, :])
                pO = ps.tile([128, 512], F32, tag="pacc_main", space="PSUM")
                for hb in range(2):
                    ptp = pst.tile([128, 128], BF16, tag="tp_ps", space="PSUM")
                    nc.tensor.transpose(out=ptp[:], in_=Pb[:, hb * 128:(hb + 1) * 128],
                                        identity=C["identb"][:])
                    PT = sb.tile([128, 128], BF16, tag="PT")
                    nc.vector.tensor_copy(out=PT[:], in_=ptp[:])
                    nc.tensor.matmul(out=pO[:, hb * 256:(hb + 1) * 256], lhsT=PT[:],
                                     rhs=C["Z1B0" if hb == 0 else "Z1B1"][:],
                                     start=True, stop=True)
                y = sb.tile([128, 512], F32, tag="y1")
                nc.vector.tensor_tensor(out=y[:], in0=pO[:], in1=C["b1t"][:], op=ALU.add)
                h1 = layer_norm_elu(nc, sb, y, C["g1t"], C["be1t"], 512, epsc)
                h1b = sb.tile([128, 512], BF16, tag="h1b")
                nc.vector.tensor_copy(out=h1b[:], in_=h1[:])
                pz = ps.tile([128, 512], F32, tag="pz", space="PSUM")
                pzb = ps.tile([128, 8], F32, tag="pzb", space="PSUM")
                for k in range(4):
                    ptp = pst.tile([128, 128], BF16, tag="tp_ps", space="PSUM")
                    nc.tensor.transpose(out=ptp[:], in_=h1b[:, k * 128:(k + 1) * 128],
                                        identity=C["identb"][:])
                    hT = sb.tile([128, 128], BF16, tag="hT")
                    nc.vector.tensor_copy(out=hT[:], in_=ptp[:])
                    nc.tensor.matmul(out=pz[:], lhsT=hT[:], rhs=C["W2s"][:, k * 520:k * 520 + 512],
                                     start=(k == 0), stop=(k == 3))
                    nc.tensor.matmul(out=pzb[:], lhsT=hT[:], rhs=C["W2s"][:, k * 520 + 512:(k + 1) * 520],
                                     start=(k == 0), stop=(k == 3))
                zs = sb.tile([128, REC2], BF16, tag="zs")
                nc.vector.tensor_copy(out=zs[:, :512], in_=pz[:])
                nc.vector.tensor_copy(out=zs[:, 512:520], in_=pzb[:])
                nc.vector.memset(zs[:, 520:], 0.0)
                nc.sync.dma_start(out=rec2_sh[r0:r0 + 128, :], in_=zs[:])
                if (g + 1) % GPC == 0:
                    k = g // GPC
                    nc.gpsimd.collective_compute(
                        "AllGather", ALU.bypass, replica_groups=[list(range(NC))],
                        ins=[rec2_sh[k * CH:(k + 1) * CH, :].opt()],
                        outs=[rec2_full[k * CHF:(k + 1) * CHF, :].opt()])

            # ================= L2 + phaseA(L3) =================
            for g in range(NG):
                r0 = g * 128
                T = Tg[g]; tA = TA[g]; tB = TB[g]
                o0 = toff[g]
                ia = sb.tile([128, T * 8], I16, tag="ia")
                nc.sync.dma_start(out=ia[:], in_=t_ig[:, o0 * 8:(o0 + T) * 8])
                idn = sb.tile([128, T * 8], I16, tag="idn")
                nc.sync.dma_start(out=idn[:], in_=t_id[:, o0 * 8:(o0 + T) * 8])
                dl = sb.tile([128, T], BF16, tag="dl")
                nc.sync.dma_start(out=dl[:], in_=t_dl[:, o0:o0 + T])
                zg = sb.tile([128, 520], BF16, tag="zg")
                nc.sync.dma_start(out=zg[:], in_=rec2_sh[r0:r0 + 128, :520])
                Gt = gb.tile([128, T, REC2], BF16, tag="G")
                emit_gather(nc, Gt, rec2_full[0:HALF, :], ia, tA, REC2)
                emit_gather(nc, Gt, rec2_full[HALF:NFULL, :], ia, tB, REC2,
                            t0=tA, i0=tA)
                arD = gb.tile([128, T, 128], BF16, tag="arD")
                emit_gather(nc, arD, rec2_sh[:, 512:640], idn, T, 128,
                            elem_step=REC2)
                S = gb.tile([128, T, 128], BF16, tag="S")
                nc.vector.tensor_tensor(
                    out=S[:, :, :],
                    in0=C["iotab"][:, None, :].to_broadcast([128, T, 128]),
                    in1=dl[:, :, None].to_broadcast([128, T, 128]),
                    op=ALU.is_equal)
                eL = sb.tile([128, T * H2], F32, tag="eL")
                nc.vector.tensor_tensor(
                    out=eL[:].rearrange("p (t h) -> p t h", h=H2),
                    in0=Gt[:, :, 512:516], in1=arD[:, :, 4:8], op=ALU.add)
                eA = sb.tile([128, T * H2], F32, tag="eA")
                nc.vector.tensor_scalar_mul(out=eA[:], in0=eL[:], scalar1=0.2)
                nc.vector.tensor_tensor(out=eA[:], in0=eL[:], in1=eA[:], op=ALU.max)
                EX = sb.tile([128, T * H2], F32, tag="EX")
                nc.scalar.activation(out=EX[:], in_=eA[:], func=ACT.Exp)
                EXb = sb.tile([128, T * H2], BF16, tag="EXb")
                nc.vector.tensor_copy(out=EXb[:], in_=EX[:])
                nc.vector.tensor_tensor(
                    out=Gt[:, :, :512].rearrange("p t (h c) -> p t h c", h=H2),
                    in0=Gt[:, :, :512].rearrange("p t (h c) -> p t h c", h=H2),
                    in1=EXb[:].rearrange("p (t h) -> p t h", h=H2)[:, :, :, None]
                        .to_broadcast([128, T, H2, C2]),
                    op=ALU.mult)
                pMain = ps.tile([128, 512], F32, tag="pacc_main", space="PSUM")
                pS = ps.tile([128, H2], F32, tag="pacc_s", space="PSUM")
                for t in range(T):
                    nc.tensor.matmul(out=pMain[:], lhsT=S[:, t, :], rhs=Gt[:, t, :512],
                                     start=(t == 0), stop=(t == T - 1))
                    nc.tensor.matmul(out=pS[:], lhsT=S[:, t, :], rhs=EXb[:, t * H2:(t + 1) * H2],
                                     start=(t == 0), stop=(t == T - 1))
                # self-loop (clamp eSl at 30: pad-row records can be large)
                eSl = sb.tile([128, H2], F32, tag="eSl")
                nc.vector.tensor_tensor(out=eSl[:], in0=zg[:, 512:516], in1=zg[:, 516:520], op=ALU.add)
                nc.vector.tensor_scalar_min(out=eSl[:], in0=eSl[:], scalar1=30.0)
                eSa = sb.tile([128, H2], F32, tag="eSa")
                nc.vector.tensor_scalar_mul(out=eSa[:], in0=eSl[:], scalar1=0.2)
                nc.vector.tensor_tensor(out=eSa[:], in0=eSl[:], in1=eSa[:], op=ALU.max)
                exS = sb.tile([128, H2], F32, tag="exS")
                nc.scalar.activation(out=exS[:], in_=eSa[:], func=ACT.Exp)
                zf = sb.tile([128, 512], F32, tag="zf")
                nc.vector.tensor_copy(out=zf[:], in_=zg[:, :512])
                selfc = sb.tile([128, 512], F32, tag="selfc")
                nc.vector.tensor_tensor(
                    out=selfc[:].rearrange("p (h c) -> p h c", h=H2),
                    in0=zf[:].rearrange("p (h c) -> p h c", h=H2),
                    in1=exS[:, :, None].to_broadcast([128, H2, C2]), op=ALU.mult)
                nc.vector.tensor_tensor(out=selfc[:], in0=pMain[:], in1=selfc[:], op=ALU.add)
                sS = sb.tile([128, H2], F32, tag="sS")
                nc.vector.tensor_tensor(out=sS[:], in0=pS[:], in1=exS[:], op=ALU.add)
                rS = sb.tile([128, H2], F32, tag="rS")
                nc.vector.reciprocal(out=rS[:], in_=sS[:])
                nc.vector.tensor_tensor(
                    out=selfc[:].rearrange("p (h c) -> p h c", h=H2),
                    in0=selfc[:].rearrange("p (h c) -> p h c", h=H2),
                    in1=rS[:, :, None].to_broadcast([128, H2, C2]), op=ALU.mult)
                nc.vector.tensor_tensor(out=selfc[:], in0=selfc[:], in1=C["b2t"][:], op=ALU.add)
                h2 = layer_norm_elu(nc, sb, selfc, C["g2t"], C["be2t"], 512, epsc)
                h2b = sb.tile([128, 512], BF16, tag="h2b")
                nc.vector.tensor_copy(out=h2b[:], in_=h2[:])
                pz3 = ps.tile([128, 18], F32, tag="pz", space="PSUM")
                for k in range(4):
                    ptp = pst.tile([128, 128], BF16, tag="tp_ps", space="PSUM")
                    nc.tensor.transpose(out=ptp[:], in_=h2b[:, k * 128:(k + 1) * 128],
                                        identity=C["identb"][:])
                    hT = sb.tile([128, 128], BF16, tag="hT")
                    nc.vector.tensor_copy(out=hT[:], in_=ptp[:])
                    nc.tensor.matmul(out=pz3[:], lhsT=hT[:], rhs=C["W3s"][:, k * 18:(k + 1) * 18],
                                     start=(k == 0), stop=(k == 3))
                z3s = sb.tile([128, REC3], BF16, tag="z3s")
                nc.vector.tensor_copy(out=z3s[:, :18], in_=pz3[:])
                nc.vector.memset(z3s[:, 18:], 0.0)
                nc.sync.dma_start(out=rec3_sh[r0:r0 + 128, :], in_=z3s[:])
                if (g + 1) % GPC == 0:
                    k = g // GPC
                    nc.gpsimd.collective_compute(
                        "AllGather", ALU.bypass, replica_groups=[list(range(NC))],
                        ins=[rec3_sh[k * CH:(k + 1) * CH, :].opt()],
                        outs=[rec3_full[k * CHF:(k + 1) * CHF, :].opt()])

            # ================= L3 + pooling =================
            pPool = pacc.tile([128, 17], F32, tag="pPool", space="PSUM")
            for g in range(NG):
                r0 = g * 128
                T = Tg[g]; tA = TA[g]; tB = TB[g]
                o0 = toff[g]
                ia = sb.tile([128, T * 8], I16, tag="ia")
                nc.sync.dma_start(out=ia[:], in_=t_ig[:, o0 * 8:(o0 + T) * 8])
                idn = sb.tile([128, T * 8], I16, tag="idn")
                nc.sync.dma_start(out=idn[:], in_=t_id[:, o0 * 8:(o0 + T) * 8])
                dl = sb.tile([128, T], BF16, tag="dl")
                nc.sync.dma_start(out=dl[:], in_=t_dl[:, o0:o0 + T])
                zg3 = sb.tile([128, 18], BF16, tag="zg3")
                nc.sync.dma_start(out=zg3[:], in_=rec3_sh[r0:r0 + 128, :18])
                bg = sb.tile([128, 1], BF16, tag="bg")
                nc.sync.dma_start(out=bg[:], in_=t_bat[r0:r0 + 128, :])
                Gt = gb.tile([128, T, REC3], BF16, tag="G3")
                emit_gather(nc, Gt, rec3_full[0:HALF, :], ia, tA, REC3)
                emit_gather(nc, Gt, rec3_full[HALF:NFULL, :], ia, tB, REC3,
                            t0=tA, i0=tA)
                arD = gb.tile([128, T, 128], BF16, tag="arD3")
                emit_gather(nc, arD, rec3_sh[:, :], idn, T, 128)
                S = gb.tile([128, T, 128], BF16, tag="S3")
                nc.vector.tensor_tensor(
                    out=S[:, :, :],
                    in0=C["iotab"][:, None, :].to_broadcast([128, T, 128]),
                    in1=dl[:, :, None].to_broadcast([128, T, 128]),
                    op=ALU.is_equal)
                eL = sb.tile([128, T], F32, tag="eL3")
                nc.vector.tensor_tensor(out=eL[:], in0=Gt[:, :, 16], in1=arD[:, :, 17], op=ALU.add)
                eA = sb.tile([128, T], F32, tag="eA3")
                nc.vector.tensor_scalar_mul(out=eA[:], in0=eL[:], scalar1=0.2)
                nc.vector.tensor_tensor(out=eA[:], in0=eL[:], in1=eA[:], op=ALU.max)
                EX = sb.tile([128, T], F32, tag="EX3")
                nc.scalar.activation(out=EX[:], in_=eA[:], func=ACT.Exp)
                EXb = sb.tile([128, T], BF16, tag="EXb3")
                nc.vector.tensor_copy(out=EXb[:], in_=EX[:])
                nc.vector.tensor_tensor(
                    out=Gt[:, :, :16], in0=Gt[:, :, :16],
                    in1=EXb[:, :, None].to_broadcast([128, T, 16]), op=ALU.mult)
                nc.vector.tensor_copy(out=Gt[:, :, 16:17],
                                      in_=EXb[:].rearrange("p (t o) -> p t o", o=1))
                pF = ps.tile([128, 17], F32, tag="pacc_main", space="PSUM")
                for t in range(T):
                    nc.tensor.matmul(out=pF[:], lhsT=S[:, t, :], rhs=Gt[:, t, :17],
                                     start=(t == 0), stop=(t == T - 1))
                eSl = sb.tile([128, 1], F32, tag="eSl3")
                nc.vector.tensor_tensor(out=eSl[:], in0=zg3[:, 16:17], in1=zg3[:, 17:18], op=ALU.add)
                nc.vector.tensor_scalar_min(out=eSl[:], in0=eSl[:], scalar1=30.0)
                eSa = sb.tile([128, 1], F32, tag="eSa3")
                nc.vector.tensor_scalar_mul(out=eSa[:], in0=eSl[:], scalar1=0.2)
                nc.vector.tensor_tensor(out=eSa[:], in0=eSl[:], in1=eSa[:], op=ALU.max)
                exS = sb.tile([128, 1], F32, tag="exS3")
                nc.scalar.activation(out=exS[:], in_=eSa[:], func=ACT.Exp)
                zf3 = sb.tile([128, 16], F32, tag="zf3")
                nc.vector.tensor_copy(out=zf3[:], in_=zg3[:, :16])
                selfc = sb.tile([128, 16], F32, tag="selfc3")
                nc.vector.tensor_scalar(out=selfc[:], in0=zf3[:], scalar1=exS[:, :1],
                                        scalar2=None, op0=ALU.mult)
                nc.vector.tensor_tensor(out=selfc[:], in0=pF[:, :16], in1=selfc[:], op=ALU.add)
                sS = sb.tile([128, 1], F32, tag="sS3")
                nc.vector.tensor_tensor(out=sS[:], in0=pF[:, 16:17], in1=exS[:], op=ALU.add)
                rS = sb.tile([128, 1], F32, tag="rS3")
                nc.vector.reciprocal(out=rS[:], in_=sS[:])
                nc.vector.tensor_scalar(out=selfc[:], in0=selfc[:], scalar1=rS[:, :1],
                                        scalar2=None, op0=ALU.mult)
                nc.vector.tensor_tensor(out=selfc[:], in0=selfc[:], in1=C["b3t"][:], op=ALU.add)
                h3 = layer_norm_elu(nc, sb, selfc, C["g3t"], C["be3t"], 16, epsc)
                OB = sb.tile([128, G], BF16, tag="OB")
                nc.vector.tensor_tensor(
                    out=OB[:], in0=C["iotab"][:, :G],
                    in1=bg[:, :1].to_broadcast([128, G]), op=ALU.is_equal)
                h3w = sb.tile([128, 17], BF16, tag="h3w")
                nc.vector.tensor_copy(out=h3w[:, :16], in_=h3[:])
                nc.vector.memset(h3w[:, 16:17], 1.0)
                nc.tensor.matmul(out=pPool[:G, :17], lhsT=OB[:], rhs=h3w[:],
                                 start=(g == 0), stop=(g == NG - 1))
            po = sb.tile([128, 17], F32, tag="po")
            nc.vector.tensor_copy(out=po[:G, :], in_=pPool[:G, :])
            nc.sync.dma_start(out=t_out[:, :], in_=po[:G, :])
    nc.finalize()
    return nc


# ======================= kernel entry =======================
_CACHE = {}


def _cache_key(meta):
    return (tuple(meta['TA']), tuple(meta['TB']))


def _hw_runner(nc, in_maps):
    from concourse.bass_utils import run_bass_kernel_spmd
    res = run_bass_kernel_spmd(nc, in_maps, core_ids=list(range(len(in_maps))))
    return res.results


def kernel(**inputs):
    consts, percore, meta, host = host_prep(inputs, N_FULL, E_FULL, G_FULL, NC_FULL)
    key = _cache_key(meta)
    if key not in _CACHE:
        _CACHE[key] = build(meta)
    nc = _CACHE[key]
    in_maps = []
    for c in range(NC_FULL):
        m = dict(consts)
        m.update(percore[c])
        in_maps.append(m)
    results = _hw_runner(nc, in_maps)
    parts = np.stack([r["part"] for r in results])
    tot = parts.sum(axis=0)
    pooled = tot[:, :16] / np.maximum(tot[:, 16:17], 1.0)
    h = np.maximum(pooled @ host['fcW1'] + host['fcb1'], 0.0)
    return (h @ host['fcW2'] + host['fcb2']).astype(np.float32)


# revision 10
# speedup vs baseline: 1.7989x; 1.4511x over previous
"""MinamoTopoModel GAT kernel: host preprocessing + Bass builder.

8-core SPMD, dst-sharded. v2 design:
  L1: cnt-histogram trick -> blockdiag matmuls (2 per group), no edge work.
  L2/L3: per-group batched dma_gather of src node records (bf16, one
         instruction per group-half instead of one indirect DMA per 128
         edges), ar[dst] via a second local dma_gather, S-matrix
         (iota-compare) PSUM scatter matmuls in bf16, segment softmax
         without max-subtraction, self-loops handled per-group directly.
  Chunked AllGathers (7 x 896 rows) publish per-shard node records between
  layers, overlapping the collective with the producing loop.
  Graph pooling -> per-core [50,17] partials; final FC on host.

rec2_full rows exceed int16 gather-index range (50176 > 32767), so edges
are split per group into two gathers against the low/high half-tables.
"""
import numpy as np
import ml_dtypes
import concourse.bacc as bacc
import concourse.bass as bass
import concourse.mybir as mybir
import concourse.tile as tile

F32 = mybir.dt.float32
BF16 = mybir.dt.bfloat16
I16 = mybir.dt.int16
AX = mybir.AxisListType
ALU = mybir.AluOpType
ACT = mybir.ActivationFunctionType
EPS = 1e-5
BF = ml_dtypes.bfloat16

N_FULL, E_FULL, G_FULL, NC_FULL = 50000, 800000, 50, 8
TILE, EMB = 32, 16
H1, C1, H2, C2, H3, C3 = 8, 64, 4, 128, 1, 16
REC2 = 640    # bf16: 512 z + 4 al + 4 ar + 120 pad  (1280B, %256==0)
REC3 = 128    # bf16: 16 z + 1 al + 1 ar + 110 pad   (256B)
NCHUNK = 1    # AllGather chunks (tile framework: single writer per Shared tensor)


def _wrap16(arr):
    """int16 idx list (len%16==0) -> [128, n/16] wrapped + replicated."""
    n = len(arr)
    w = arr.reshape(n // 16, 16).T            # [16, n/16]
    return np.tile(w, (8, 1)).astype(np.int16)


def host_prep(inputs, N, E, G, NC):
    x = np.asarray(inputs['x']).astype(np.int64)
    ei = np.asarray(inputs['edge_index']).astype(np.int64)
    batch = np.asarray(inputs['batch']).astype(np.int64)
    emb = np.asarray(inputs['emb'], np.float32)
    W1 = np.asarray(inputs['W1'], np.float32)
    as1 = np.asarray(inputs['a_src1'], np.float32); ad1 = np.asarray(inputs['a_dst1'], np.float32)
    b1 = np.asarray(inputs['b1'], np.float32)
    g1 = np.asarray(inputs['g1'], np.float32); be1 = np.asarray(inputs['be1'], np.float32)
    W2 = np.asarray(inputs['W2'], np.float32)
    as2 = np.asarray(inputs['a_src2'], np.float32); ad2 = np.asarray(inputs['a_dst2'], np.float32)
    b2 = np.asarray(inputs['b2'], np.float32)
    g2 = np.asarray(inputs['g2'], np.float32); be2 = np.asarray(inputs['be2'], np.float32)
    W3 = np.asarray(inputs['W3'], np.float32)
    as3 = np.asarray(inputs['a_src3'], np.float32); ad3 = np.asarray(inputs['a_dst3'], np.float32)
    b3 = np.asarray(inputs['b3'], np.float32)
    g3 = np.asarray(inputs['g3'], np.float32); be3 = np.asarray(inputs['be3'], np.float32)

    NPC = N // NC                      # 6250
    NG = (NPC + 127) // 128            # 49
    NPCP = NG * 128                    # 6272
    CH = NPCP // NCHUNK                # 896 rows per AllGather chunk
    CHF = CH * NC                      # 7168 rows per full chunk
    NFULL = NC * NPCP                  # 50176
    HALF = NFULL // 2                  # 25088

    # ---- L1 tables (cnt trick) ----
    z1 = emb @ W1                                     # [32, 512]
    z1h = z1.reshape(TILE, H1, C1)
    al1t = np.einsum('thc,hc->th', z1h, as1)          # [32,8]
    ar1t = np.einsum('thc,hc->th', z1h, ad1)
    ee = al1t.T[None, :, :] + ar1t[:, :, None]        # [xd=32, h=8, t=32]
    ee = np.where(ee > 0, ee, 0.2 * ee)
    E_tab = np.exp(ee).astype(np.float32)             # [32, 8, 32]

    src_all = np.concatenate([ei[0], np.arange(N)])
    dst_all = np.concatenate([ei[1], np.arange(N)])
    xs_all = x[src_all]
    cnt = np.zeros((N, TILE), np.float32)
    np.add.at(cnt, (dst_all, xs_all), 1.0)

    # blockdiag Z1B: Z1B[hb][h*32+t, h*64+c] = z1[t, (hb*4+h)*64+c]
    Z1B = np.zeros((2, 128, 256), np.float32)
    for hb in range(2):
        for h in range(4):
            Z1B[hb, h * 32:(h + 1) * 32, h * 64:(h + 1) * 64] = \
                z1[:, (hb * 4 + h) * 64:(hb * 4 + h + 1) * 64]

    # ---- weight tables: W' = [W | W@As | W@Ad] ----
    def wprime(W, a_s, a_d, H, C):
        As = np.zeros((H * C, H), np.float32)
        Ad = np.zeros((H * C, H), np.float32)
        for h in range(H):
            As[h * C:(h + 1) * C, h] = a_s[h]
            Ad[h * C:(h + 1) * C, h] = a_d[h]
        return np.concatenate([W, W @ As, W @ Ad], axis=1)  # [512, H*C+2H]

    W2p = wprime(W2, as2, ad2, H2, C2)   # [512, 520]
    W3p = wprime(W3, as3, ad3, H3, C3)   # [512, 18]
    W2s = np.concatenate([W2p[k * 128:(k + 1) * 128] for k in range(4)], axis=1)  # [128, 4*520]
    W3s = np.concatenate([W3p[k * 128:(k + 1) * 128] for k in range(4)], axis=1)  # [128, 4*18]

    def bc(v, F):
        t = np.zeros((128, F), np.float32); t[:, :] = v[None, :F]; return t

    iota = np.tile(np.arange(128, dtype=np.float32), (128, 1))
    consts = dict(
        Z1B0=Z1B[0].astype(BF), Z1B1=Z1B[1].astype(BF),
        W2s=W2s.astype(BF), W3s=W3s.astype(BF),
        b1t=bc(b1, 512), g1t=bc(g1, 512), be1t=bc(be1, 512),
        b2t=bc(b2, 512), g2t=bc(g2, 512), be2t=bc(be2, 512),
        b3t=bc(b3, 16), g3t=bc(g3, 16), be3t=bc(be3, 16),
        iotab=iota.astype(BF),
        identb=np.eye(128, dtype=BF),
    )

    # ---- per-core edge bucketing ----
    es, ed = ei[0], ei[1]
    # global gathered-row of a node (chunked AllGather layout)
    c0 = es // NPC
    r = es % NPC
    kk = r // CH
    grow = kk * CHF + c0 * CH + (r - kk * CH)       # [E]
    half = (grow >= HALF).astype(np.int64)

    core_of = ed // NPC
    rd = ed % NPC
    grp_of = rd // 128
    dstloc = rd % 128

    # counts per (core, group, half)
    cnts = np.zeros((NC, NG, 2), np.int64)
    np.add.at(cnts, (core_of, grp_of, half), 1)
    TA = np.maximum(1, (cnts[:, :, 0].max(axis=0) + 127) // 128)  # [NG]
    TB = np.maximum(1, (cnts[:, :, 1].max(axis=0) + 127) // 128)
    Tg = TA + TB
    toff = np.concatenate([[0], np.cumsum(Tg)]).astype(np.int64)
    NTT = int(toff[-1])

    order = np.lexsort((es, half, grp_of, core_of))
    es_s = es[order]; half_s = half[order]
    core_s, grp_s = core_of[order], grp_of[order]
    grow_s = grow[order]; dstloc_s = dstloc[order]; rd_s = rd[order]
    flat = (core_s * NG + grp_s) * 2 + half_s
    bounds = np.searchsorted(flat, np.arange(NC * NG * 2 + 1))

    percore = []
    for c in range(NC):
        idxg = np.zeros((NTT * 128,), np.int64)
        idxd = np.zeros((NTT * 128,), np.int64)
        dlf = np.full((NTT * 128,), 200.0, np.float32)
        for g in range(NG):
            o = int(toff[g]) * 128
            for h in range(2):
                f = (c * NG + g) * 2 + h
                s, e = bounds[f], bounds[f + 1]
                n = e - s
                cap = int((TA if h == 0 else TB)[g]) * 128
                assert n <= cap, (c, g, h, n, cap)
                oo = o + (int(TA[g]) * 128 if h else 0)
                idxg[oo:oo + n] = grow_s[s:e] - HALF * h
                idxd[oo:oo + n] = rd_s[s:e]
                dlf[oo:oo + n] = dstloc_s[s:e]
        idxg16 = _wrap16(idxg.astype(np.int16))          # [128, NTT*8]
        idxd16 = _wrap16(idxd.astype(np.int16))
        dls = np.ascontiguousarray(
            dlf.reshape(NTT, 128).T).astype(BF)          # [128, NTT]

        lo, hi = c * NPC, (c + 1) * NPC
        cntc = np.zeros((NPCP, TILE), np.float32)
        cntc[:NPC] = cnt[lo:hi]
        cntc[NPC:, 0] = 1.0
        Ec = np.zeros((NPCP, H1 * TILE), np.float32)
        Ec[:NPC] = E_tab[x[lo:hi]].reshape(NPC, H1 * TILE)
        Ec[NPC:] = 1.0
        batchc = np.full((NPCP, 1), 200.0, np.float32)
        batchc[:NPC, 0] = batch[lo:hi]
        percore.append(dict(
            cntc=cntc, Ec=Ec, batchc=batchc.astype(BF),
            idxg16=idxg16, idxd16=idxd16, dls=dls,
        ))

    meta = dict(N=N, E=E, G=G, NC=NC, NPC=NPC, NG=NG, NPCP=NPCP,
                CH=CH, CHF=CHF, NFULL=NFULL, HALF=HALF,
                TA=TA.tolist(), TB=TB.tolist(), Tg=Tg.tolist(),
                toff=toff.tolist())
    host = dict(fcW1=np.asarray(inputs['fcW1'], np.float32),
                fcb1=np.asarray(inputs['fcb1'], np.float32),
                fcW2=np.asarray(inputs['fcW2'], np.float32),
                fcb2=np.asarray(inputs['fcb2'], np.float32))
    return consts, percore, meta, host


_QCTR = [0]
NQ = 4  # SWDGE queues


def emit_gather(nc, out3, table, idxt, ntiles, elem, elem_step=None, t0=0, i0=0):
    """dma_gather capped at 1024 idxs (8 tiles) per instruction, round-robin
    across SWDGE queues so descriptor-gen overlaps DMA drains.

    out3: [128, T, elem] SBUF tile (writes tiles [t0, t0+ntiles));
    idxt: [128, ncols] int16 SBUF tile (reads cols [i0*8, (i0+ntiles)*8)).
    """
    CAP = 8
    t = 0
    while t < ntiles:
        n = min(CAP, ntiles - t)
        q = _QCTR[0] % NQ
        _QCTR[0] += 1
        nc.gpsimd.dma_gather(
            out3[:, t0 + t:t0 + t + n, :], table,
            idxt[:, (i0 + t) * 8:(i0 + t + n) * 8],
            n * 128, n * 128, elem, elem_step=elem_step, queue_num=q)
        t += n


def layer_norm_elu(nc, pool, y, g_t, be_t, F, epsc):
    """In SBUF f32: y [128,F] -> elu(LN(y)*g+be). Returns new tile."""
    s1 = pool.tile([128, 1], F32, tag="ln_s1")
    nc.vector.tensor_reduce(out=s1[:], in_=y[:], axis=AX.X, op=ALU.add)
    m2 = pool.tile([128, 1], F32, tag="ln_m2")
    nc.vector.tensor_scalar_mul(out=m2[:], in0=s1[:], scalar1=-1.0 / F)
    sq = pool.tile([128, F], F32, tag="ln_sq")
    ss = pool.tile([128, 1], F32, tag="ln_ss")
    nc.scalar.activation(out=sq[:], in_=y[:], func=ACT.Square, bias=m2[:, :1],
                         accum_out=ss[:])
    sd = pool.tile([128, 1], F32, tag="ln_sd")
    nc.scalar.activation(out=sd[:], in_=ss[:], func=ACT.Sqrt, bias=epsc[:, :1], scale=1.0 / F)
    rs = pool.tile([128, 1], F32, tag="ln_rs")
    nc.vector.reciprocal(out=rs[:], in_=sd[:])
    nc.vector.tensor_scalar(out=y[:], in0=y[:], scalar1=m2[:, :1], scalar2=rs[:, :1],
                            op0=ALU.add, op1=ALU.mult)
    nc.vector.tensor_tensor(out=y[:], in0=y[:], in1=g_t[:, :F], op=ALU.mult)
    nc.vector.tensor_tensor(out=y[:], in0=y[:], in1=be_t[:, :F], op=ALU.add)
    nc.vector.tensor_scalar_min(out=sq[:], in0=y[:], scalar1=0.0)
    nc.scalar.activation(out=sq[:], in_=sq[:], func=ACT.Exp)
    h = pool.tile([128, F], F32, tag="elu_h")
    nc.vector.tensor_scalar(out=h[:], in0=y[:], scalar1=0.0, scalar2=-1.0,
                            op0=ALU.max, op1=ALU.add)
    nc.vector.tensor_tensor(out=h[:], in0=h[:], in1=sq[:], op=ALU.add)
    return h


def build(meta):
    NC, NG, NPCP = meta['NC'], meta['NG'], meta['NPCP']
    TA, TB, Tg, toff = meta['TA'], meta['TB'], meta['Tg'], meta['toff']
    NTT = toff[-1]
    CH, NFULL, HALF = meta['CH'], meta['NFULL'], meta['HALF']
    CHF = meta['CHF']
    G = meta['G']

    nc = bacc.Bacc("TRN2", num_devices=NC, num_swdge_queues=NQ)
    _QCTR[0] = 0
    t_cnt = nc.dram_tensor("cntc", [NPCP, TILE], F32, kind="ExternalInput")
    t_E = nc.dram_tensor("Ec", [NPCP, H1 * TILE], F32, kind="ExternalInput")
    t_bat = nc.dram_tensor("batchc", [NPCP, 1], BF16, kind="ExternalInput")
    t_ig = nc.dram_tensor("idxg16", [128, NTT * 8], I16, kind="ExternalInput")
    t_id = nc.dram_tensor("idxd16", [128, NTT * 8], I16, kind="ExternalInput")
    t_dl = nc.dram_tensor("dls", [128, NTT], BF16, kind="ExternalInput")
    t_Z0 = nc.dram_tensor("Z1B0", [128, 256], BF16, kind="ExternalInput")
    t_Z1 = nc.dram_tensor("Z1B1", [128, 256], BF16, kind="ExternalInput")
    t_W2s = nc.dram_tensor("W2s", [128, 4 * 520], BF16, kind="ExternalInput")
    t_W3s = nc.dram_tensor("W3s", [128, 4 * 18], BF16, kind="ExternalInput")
    cn = {}
    for nm, sh in [("b1t", 512), ("g1t", 512), ("be1t", 512), ("b2t", 512),
                   ("g2t", 512), ("be2t", 512), ("b3t", 16), ("g3t", 16), ("be3t", 16)]:
        cn[nm] = nc.dram_tensor(nm, [128, sh], F32, kind="ExternalInput")
    t_iob = nc.dram_tensor("iotab", [128, 128], BF16, kind="ExternalInput")
    t_idb = nc.dram_tensor("identb", [128, 128], BF16, kind="ExternalInput")
    t_out = nc.dram_tensor("part", [G, 17], F32, kind="ExternalOutput")

    with tile.TileContext(nc) as tc:
        with tc.tile_pool(name="const", bufs=1) as cp, \
             tc.tile_pool(name="sb", bufs=2) as sb, \
             tc.tile_pool(name="gbuf", bufs=2) as gb, \
             tc.tile_pool(name="ps", bufs=1, space="PSUM") as ps, \
             tc.tile_pool(name="pst", bufs=2, space="PSUM") as pst, \
             tc.tile_pool(name="pacc", bufs=1, space="PSUM") as pacc, \
             tc.tile_pool(name="dram", bufs=1, space="DRAM") as dp:

            # ---- const loads ----
            C = {}
            for nm, src, shp, dt in [
                    ("iotab", t_iob, [128, 128], BF16),
                    ("identb", t_idb, [128, 128], BF16),
                    ("Z1B0", t_Z0, [128, 256], BF16),
                    ("Z1B1", t_Z1, [128, 256], BF16),
                    ("W2s", t_W2s, [128, 4 * 520], BF16),
                    ("W3s", t_W3s, [128, 4 * 18], BF16)]:
                C[nm] = cp.tile(shp, dt, tag="c_" + nm, name="c_" + nm)
                nc.sync.dma_start(out=C[nm][:], in_=src[:])
            for nm in cn:
                F = 512 if nm[-2] != '3' else 16
                C[nm] = cp.tile([128, F], F32, tag="c_" + nm, name="c_" + nm)
                nc.sync.dma_start(out=C[nm][:], in_=cn[nm][:])
            epsc = cp.tile([128, 1], F32, name="epsc")
            nc.vector.memset(epsc[:], EPS)

            rec2_sh = dp.tile([NPCP, REC2], BF16)
            rec2_full = dp.tile([NFULL, REC2], BF16, addr_space="Shared")
            rec3_sh = dp.tile([NPCP, REC3], BF16)
            rec3_full = dp.tile([NFULL, REC3], BF16, addr_space="Shared")

            GPC = NG // NCHUNK  # groups per AllGather chunk (7)

            # ================= L1 + phaseA(L2) =================
            for g in range(NG):
                r0 = g * 128
                cg = sb.tile([128, TILE], F32, tag="cg")
                nc.sync.dma_start(out=cg[:], in_=t_cnt[r0:r0 + 128, :])
                Eg = sb.tile([128, H1, TILE], F32, tag="Eg")
                nc.sync.dma_start(out=Eg[:, :, :], in_=t_E[r0:r0 + 128, :].rearrange("p (h t) -> p h t", h=H1))
                M = sb.tile([128, H1, TILE], F32, tag="M")
                nc.vector.tensor_tensor(out=M[:, :, :], in0=Eg[:, :, :],
                                        in1=cg[:, None, :].to_broadcast([128, H1, TILE]),
                                        op=ALU.mult)
                s = sb.tile([128, H1], F32, tag="s")
                nc.vector.tensor_reduce(out=s[:], in_=M[:, :, :], axis=AX.X, op=ALU.add)
                rs = sb.tile([128, H1], F32, tag="rs")
                nc.vector.reciprocal(out=rs[:], in_=s[:])
                nc.vector.tensor_tensor(out=M[:, :, :], in0=M[:, :, :],
                                        in1=rs[:, :, None].to_broadcast([128, H1, TILE]),
                                        op=ALU.mult)
                Pb = sb.tile([128, 256], BF16, tag="Pb")
                nc.vector.tensor_copy(out=Pb[:].rearrange("p (h t) -> p h t", h=H1),
                                      in_=M[:, :, :])
                pO = ps.tile([128, 512], F32, tag="pacc_main", space="PSUM")
                for hb in range(2):
                    ptp = pst.tile([128, 128], BF16, tag="tp_ps", space="PSUM")
                    nc.tensor.transpose(out=ptp[:], in_=Pb[:, hb * 128:(hb + 1) * 128],
                                        identity=C["identb"][:])
                    PT = sb.tile([128, 128], BF16, tag="PT")
                    nc.vector.tensor_copy(out=PT[:], in_=ptp[:])
                    nc.tensor.matmul(out=pO[:, hb * 256:(hb + 1) * 256], lhsT=PT[:],
                                     rhs=C["Z1B0" if hb == 0 else "Z1B1"][:],
                                     start=True, stop=True)
                y = sb.tile([128, 512], F32, tag="y1")
                nc.vector.tensor_tensor(out=y[:], in0=pO[:], in1=C["b1t"][:], op=ALU.add)
                h1 = layer_norm_elu(nc, sb, y, C["g1t"], C["be1t"], 512, epsc)
                h1b = sb.tile([128, 512], BF16, tag="h1b")
                nc.vector.tensor_copy(out=h1b[:], in_=h1[:])
                pz = ps.tile([128, 512], F32, tag="pz", space="PSUM")
                pzb = ps.tile([128, 8], F32, tag="pzb", space="PSUM")
                for k in range(4):
                    ptp = pst.tile([128, 128], BF16, tag="tp_ps", space="PSUM")
                    nc.tensor.transpose(out=ptp[:], in_=h1b[:, k * 128:(k + 1) * 128],
                                        identity=C["identb"][:])
                    hT = sb.tile([128, 128], BF16, tag="hT")
                    nc.vector.tensor_copy(out=hT[:], in_=ptp[:])
                    nc.tensor.matmul(out=pz[:], lhsT=hT[:], rhs=C["W2s"][:, k * 520:k * 520 + 512],
                                     start=(k == 0), stop=(k == 3))
                    nc.tensor.matmul(out=pzb[:], lhsT=hT[:], rhs=C["W2s"][:, k * 520 + 512:(k + 1) * 520],
                                     start=(k == 0), stop=(k == 3))
                zs = sb.tile([128, REC2], BF16, tag="zs")
                nc.vector.tensor_copy(out=zs[:, :512], in_=pz[:])
                nc.vector.tensor_copy(out=zs[:, 512:520], in_=pzb[:])
                nc.vector.memset(zs[:, 520:], 0.0)
                nc.sync.dma_start(out=rec2_sh[r0:r0 + 128, :], in_=zs[:])
                if (g + 1) % GPC == 0:
                    k = g // GPC
                    nc.gpsimd.collective_compute(
                        "AllGather", ALU.bypass, replica_groups=[list(range(NC))],
                        ins=[rec2_sh[k * CH:(k + 1) * CH, :].opt()],
                        outs=[rec2_full[k * CHF:(k + 1) * CHF, :].opt()])

            # ================= L2 + phaseA(L3) =================
            for g in range(NG):
                r0 = g * 128
                T = Tg[g]; tA = TA[g]; tB = TB[g]
                o0 = toff[g]
                ia = sb.tile([128, T * 8], I16, tag="ia")
                nc.sync.dma_start(out=ia[:], in_=t_ig[:, o0 * 8:(o0 + T) * 8])
                idn = sb.tile([128, T * 8], I16, tag="idn")
                nc.sync.dma_start(out=idn[:], in_=t_id[:, o0 * 8:(o0 + T) * 8])
                dl = sb.tile([128, T], BF16, tag="dl")
                nc.sync.dma_start(out=dl[:], in_=t_dl[:, o0:o0 + T])
                zg = sb.tile([128, 520], BF16, tag="zg")
                nc.sync.dma_start(out=zg[:], in_=rec2_sh[r0:r0 + 128, :520])
                Gt = gb.tile([128, T, REC2], BF16, tag="G")
                emit_gather(nc, Gt, rec2_full[0:HALF, :], ia, tA, REC2)
                emit_gather(nc, Gt, rec2_full[HALF:NFULL, :], ia, tB, REC2,
                            t0=tA, i0=tA)
                arD = gb.tile([128, T, 128], BF16, tag="arD")
                emit_gather(nc, arD, rec2_sh[:, 512:640], idn, T, 128,
                            elem_step=REC2)
                S = gb.tile([128, T, 128], BF16, tag="S")
                nc.vector.tensor_tensor(
                    out=S[:, :, :],
                    in0=C["iotab"][:, None, :].to_broadcast([128, T, 128]),
                    in1=dl[:, :, None].to_broadcast([128, T, 128]),
                    op=ALU.is_equal)
                eL = sb.tile([128, T * H2], F32, tag="eL")
                nc.vector.tensor_tensor(
                    out=eL[:].rearrange("p (t h) -> p t h", h=H2),
                    in0=Gt[:, :, 512:516], in1=arD[:, :, 4:8], op=ALU.add)
                eA = sb.tile([128, T * H2], F32, tag="eA")
                nc.vector.tensor_scalar_mul(out=eA[:], in0=eL[:], scalar1=0.2)
                nc.vector.tensor_tensor(out=eA[:], in0=eL[:], in1=eA[:], op=ALU.max)
                EX = sb.tile([128, T * H2], F32, tag="EX")
                nc.scalar.activation(out=EX[:], in_=eA[:], func=ACT.Exp)
                EXb = sb.tile([128, T * H2], BF16, tag="EXb")
                nc.vector.tensor_copy(out=EXb[:], in_=EX[:])
                nc.vector.tensor_tensor(
                    out=Gt[:, :, :512].rearrange("p t (h c) -> p t h c", h=H2),
                    in0=Gt[:, :, :512].rearrange("p t (h c) -> p t h c", h=H2),
                    in1=EXb[:].rearrange("p (t h) -> p t h", h=H2)[:, :, :, None]
                        .to_broadcast([128, T, H2, C2]),
                    op=ALU.mult)
                pMain = ps.tile([128, 512], F32, tag="pacc_main", space="PSUM")
                pS = ps.tile([128, H2], F32, tag="pacc_s", space="PSUM")
                for t in range(T):
                    nc.tensor.matmul(out=pMain[:], lhsT=S[:, t, :], rhs=Gt[:, t, :512],
                                     start=(t == 0), stop=(t == T - 1))
                    nc.tensor.matmul(out=pS[:], lhsT=S[:, t, :], rhs=EXb[:, t * H2:(t + 1) * H2],
                                     start=(t == 0), stop=(t == T - 1))
                # self-loop (clamp eSl at 30: pad-row records can be large)
                eSl = sb.tile([128, H2], F32, tag="eSl")
                nc.vector.tensor_tensor(out=eSl[:], in0=zg[:, 512:516], in1=zg[:, 516:520], op=ALU.add)
                nc.vector.tensor_scalar_min(out=eSl[:], in0=eSl[:], scalar1=30.0)
                eSa = sb.tile([128, H2], F32, tag="eSa")
                nc.vector.tensor_scalar_mul(out=eSa[:], in0=eSl[:], scalar1=0.2)
                nc.vector.tensor_tensor(out=eSa[:], in0=eSl[:], in1=eSa[:], op=ALU.max)
                exS = sb.tile([128, H2], F32, tag="exS")
                nc.scalar.activation(out=exS[:], in_=eSa[:], func=ACT.Exp)
                zf = sb.tile([128, 512], F32, tag="zf")
                nc.vector.tensor_copy(out=zf[:], in_=zg[:, :512])
                selfc = sb.tile([128, 512], F32, tag="selfc")
                nc.vector.tensor_tensor(
                    out=selfc[:].rearrange("p (h c) -> p h c", h=H2),
                    in0=zf[:].rearrange("p (h c) -> p h c", h=H2),
                    in1=exS[:, :, None].to_broadcast([128, H2, C2]), op=ALU.mult)
                nc.vector.tensor_tensor(out=selfc[:], in0=pMain[:], in1=selfc[:], op=ALU.add)
                sS = sb.tile([128, H2], F32, tag="sS")
                nc.vector.tensor_tensor(out=sS[:], in0=pS[:], in1=exS[:], op=ALU.add)
                rS = sb.tile([128, H2], F32, tag="rS")
                nc.vector.reciprocal(out=rS[:], in_=sS[:])
                nc.vector.tensor_tensor(
                    out=selfc[:].rearrange("p (h c) -> p h c", h=H2),
                    in0=selfc[:].rearrange("p (h c) -> p h c", h=H2),
                    in1=rS[:, :, None].to_broadcast([128, H2, C2]), op=ALU.mult)
                nc.vector.tensor_tensor(out=selfc[:], in0=selfc[:], in1=C["b2t"][:], op=ALU.add)
                h2 = layer_norm_elu(nc, sb, selfc, C["g2t"], C["be2t"], 512, epsc)
                h2b = sb.tile([128, 512], BF16, tag="h2b")
                nc.vector.tensor_copy(out=h2b[:], in_=h2[:])
                pz3 = ps.tile([128, 18], F32, tag="pz", space="PSUM")
                for k in range(4):
                    ptp = pst.tile([128, 128], BF16, tag="tp_ps", space="PSUM")
                    nc.tensor.transpose(out=ptp[:], in_=h2b[:, k * 128:(k + 1) * 128],
                                        identity=C["identb"][:])
                    hT = sb.tile([128, 128], BF16, tag="hT")
                    nc.vector.tensor_copy(out=hT[:], in_=ptp[:])
                    nc.tensor.matmul(out=pz3[:], lhsT=hT[:], rhs=C["W3s"][:, k * 18:(k + 1) * 18],
                                     start=(k == 0), stop=(k == 3))
                z3s = sb.tile([128, REC3], BF16, tag="z3s")
                nc.vector.tensor_copy(out=z3s[:, :18], in_=pz3[:])
                nc.vector.memset(z3s[:, 18:], 0.0)
                nc.sync.dma_start(out=rec3_sh[r0:r0 + 128, :], in_=z3s[:])
                if (g + 1) % GPC == 0:
                    k = g // GPC
                    nc.gpsimd.collective_compute(
                        "AllGather", ALU.bypass, replica_groups=[list(range(NC))],
                        ins=[rec3_sh[k * CH:(k + 1) * CH, :].opt()],
                        outs=[rec3_full[k * CHF:(k + 1) * CHF, :].opt()])

            # ================= L3 + pooling =================
            pPool = pacc.tile([128, 17], F32, tag="pPool", space="PSUM")
            for g in range(NG):
                r0 = g * 128
                T = Tg[g]; tA = TA[g]; tB = TB[g]
                o0 = toff[g]
                ia = sb.tile([128, T * 8], I16, tag="ia")
                nc.sync.dma_start(out=ia[:], in_=t_ig[:, o0 * 8:(o0 + T) * 8])
                idn = sb.tile([128, T * 8], I16, tag="idn")
                nc.sync.dma_start(out=idn[:], in_=t_id[:, o0 * 8:(o0 + T) * 8])
                dl = sb.tile([128, T], BF16, tag="dl")
                nc.sync.dma_start(out=dl[:], in_=t_dl[:, o0:o0 + T])
                zg3 = sb.tile([128, 18], BF16, tag="zg3")
                nc.sync.dma_start(out=zg3[:], in_=rec3_sh[r0:r0 + 128, :18])
                bg = sb.tile([128, 1], BF16, tag="bg")
                nc.sync.dma_start(out=bg[:], in_=t_bat[r0:r0 + 128, :])
                Gt = gb.tile([128, T, REC3], BF16, tag="G3")
                emit_gather(nc, Gt, rec3_full[0:HALF, :], ia, tA, REC3)
                emit_gather(nc, Gt, rec3_full[HALF:NFULL, :], ia, tB, REC3,
                            t0=tA, i0=tA)
                arD = gb.tile([128, T, 128], BF16, tag="arD3")
                emit_gather(nc, arD, rec3_sh[:, :], idn, T, 128)
                S = gb.tile([128, T, 128], BF16, tag="S3")
                nc.vector.tensor_tensor(
                    out=S[:, :, :],
                    in0=C["iotab"][:, None, :].to_broadcast([128, T, 128]),
                    in1=dl[:, :, None].to_broadcast([128, T, 128]),
                    op=ALU.is_equal)
                eL = sb.tile([128, T], F32, tag="eL3")
                nc.vector.tensor_tensor(out=eL[:], in0=Gt[:, :, 16], in1=arD[:, :, 17], op=ALU.add)
                eA = sb.tile([128, T], F32, tag="eA3")
                nc.vector.tensor_scalar_mul(out=eA[:], in0=eL[:], scalar1=0.2)
                nc.vector.tensor_tensor(out=eA[:], in0=eL[:], in1=eA[:], op=ALU.max)
                EX = sb.tile([128, T], F32, tag="EX3")
                nc.scalar.activation(out=EX[:], in_=eA[:], func=ACT.Exp)
                EXb = sb.tile([128, T], BF16, tag="EXb3")
                nc.vector.tensor_copy(out=EXb[:], in_=EX[:])
                nc.vector.tensor_tensor(
                    out=Gt[:, :, :16], in0=Gt[:, :, :16],
                    in1=EXb[:, :, None].to_broadcast([128, T, 16]), op=ALU.mult)
                nc.vector.tensor_copy(out=Gt[:, :, 16:17],
                                      in_=EXb[:].rearrange("p (t o) -> p t o", o=1))
                pF = ps.tile([128, 17], F32, tag="pacc_main", space="PSUM")
                for t in range(T):
                    nc.tensor.matmul(out=pF[:], lhsT=S[:, t, :], rhs=Gt[:, t, :17],
                                     start=(t == 0), stop=(t == T - 1))
                eSl = sb.tile([128, 1], F32, tag="eSl3")
                nc.vector.tensor_tensor(out=eSl[:], in0=zg3[:, 16:17], in1=zg3[:, 17:18], op=ALU.add)
                nc.vector.tensor_scalar_min(out=eSl[:], in0=eSl[:], scalar1=30.0)
                eSa = sb.tile([128, 1], F32, tag="eSa3")
                nc.vector.tensor_scalar_mul(out=eSa[:], in0=eSl[:], scalar1=0.2)
                nc.vector.tensor_tensor(out=eSa[:], in0=eSl[:], in1=eSa[:], op=ALU.max)
                exS = sb.tile([128, 1], F32, tag="exS3")
                nc.scalar.activation(out=exS[:], in_=eSa[:], func=ACT.Exp)
                zf3 = sb.tile([128, 16], F32, tag="zf3")
                nc.vector.tensor_copy(out=zf3[:], in_=zg3[:, :16])
                selfc = sb.tile([128, 16], F32, tag="selfc3")
                nc.vector.tensor_scalar(out=selfc[:], in0=zf3[:], scalar1=exS[:, :1],
                                        scalar2=None, op0=ALU.mult)
                nc.vector.tensor_tensor(out=selfc[:], in0=pF[:, :16], in1=selfc[:], op=ALU.add)
                sS = sb.tile([128, 1], F32, tag="sS3")
                nc.vector.tensor_tensor(out=sS[:], in0=pF[:, 16:17], in1=exS[:], op=ALU.add)
                rS = sb.tile([128, 1], F32, tag="rS3")
                nc.vector.reciprocal(out=rS[:], in_=sS[:])
                nc.vector.tensor_scalar(out=selfc[:], in0=selfc[:], scalar1=rS[:, :1],
                                        scalar2=None, op0=ALU.mult)
                nc.vector.tensor_tensor(out=selfc[:], in0=selfc[:], in1=C["b3t"][:], op=ALU.add)
                h3 = layer_norm_elu(nc, sb, selfc, C["g3t"], C["be3t"], 16, epsc)
                OB = sb.tile([128, G], BF16, tag="OB")
                nc.vector.tensor_tensor(
                    out=OB[:], in0=C["iotab"][:, :G],
                    in1=bg[:, :1].to_broadcast([128, G]), op=ALU.is_equal)
                h3w = sb.tile([128, 17], BF16, tag="h3w")
                nc.vector.tensor_copy(out=h3w[:, :16], in_=h3[:])
                nc.vector.memset(h3w[:, 16:17], 1.0)
                nc.tensor.matmul(out=pPool[:G, :17], lhsT=OB[:], rhs=h3w[:],
                                 start=(g == 0), stop=(g == NG - 1))
            po = sb.tile([128, 17], F32, tag="po")
            nc.vector.tensor_copy(out=po[:G, :], in_=pPool[:G, :])
            nc.sync.dma_start(out=t_out[:, :], in_=po[:G, :])
    nc.finalize()
    return nc


# ======================= kernel entry =======================
_CACHE = {}


def _cache_key(meta):
    return (tuple(meta['TA']), tuple(meta['TB']))


def _hw_runner(nc, in_maps):
    from concourse.bass_utils import run_bass_kernel_spmd
    res = run_bass_kernel_spmd(nc, in_maps, core_ids=list(range(len(in_maps))))
    return res.results


def kernel(**inputs):
    consts, percore, meta, host = host_prep(inputs, N_FULL, E_FULL, G_FULL, NC_FULL)
    key = _cache_key(meta)
    if key not in _CACHE:
        _CACHE[key] = build(meta)
    nc = _CACHE[key]
    in_maps = []
    for c in range(NC_FULL):
        m = dict(consts)
        m.update(percore[c])
        in_maps.append(m)
    results = _hw_runner(nc, in_maps)
    parts = np.stack([r["part"] for r in results])
    tot = parts.sum(axis=0)
    pooled = tot[:, :16] / np.maximum(tot[:, 16:17], 1.0)
    h = np.maximum(pooled @ host['fcW1'] + host['fcb1'], 0.0)
    return (h @ host['fcW2'] + host['fcb2']).astype(np.float32)


# revision 11
# speedup vs baseline: 2.3533x; 1.3082x over previous
"""MinamoTopoModel GAT kernel: host preprocessing + Bass builder.

8-core SPMD, dst-sharded. v3 design:
  L1: cnt-histogram trick -> blockdiag matmuls (2 per group), bf16.
  L2/L3: batched dma_gather of src node records (bf16, <=1024 idxs per
         instruction over 4 SWDGE queues), host-precomputed scatter
         matrices S (edge-slot x dstloc one-hot) and their transposes ST.
         Per tile: pAR = ST_t^T @ ar-table gives per-edge ar[dst] (no
         second gather, no on-chip transposes); segment softmax without
         max-subtraction; PSUM scatter matmuls in bf16; self-loops
         handled per-group directly.
  AllGather (bf16) publishes per-shard node records between layers.
  Graph pooling via host-precomputed batch one-hot; final FC on host.

rec2_full rows exceed int16 gather-index range (50176 > 32767), so edges
are split per group into two gathers against the low/high half-tables.
"""
import numpy as np
import ml_dtypes
import concourse.bacc as bacc
import concourse.bass as bass
import concourse.mybir as mybir
import concourse.tile as tile

F32 = mybir.dt.float32
BF16 = mybir.dt.bfloat16
I16 = mybir.dt.int16
AX = mybir.AxisListType
ALU = mybir.AluOpType
ACT = mybir.ActivationFunctionType
EPS = 1e-5
BF = ml_dtypes.bfloat16

N_FULL, E_FULL, G_FULL, NC_FULL = 50000, 800000, 50, 8
TILE, EMB = 32, 16
H1, C1, H2, C2, H3, C3 = 8, 64, 4, 128, 1, 16
REC2 = 640    # bf16: 512 z + 4 al + 4 ar + 120 pad  (1280B, %256==0)
REC3 = 128    # bf16: 16 z + 1 al + 1 ar + 110 pad   (256B)
NCHUNK = 1
NQ = 4        # SWDGE queues
_QCTR = [0]


def _wrap16(arr):
    """int16 idx list (len%16==0) -> [128, n/16] wrapped + replicated."""
    n = len(arr)
    w = arr.reshape(n // 16, 16).T            # [16, n/16]
    return np.tile(w, (8, 1)).astype(np.int16)


def host_prep(inputs, N, E, G, NC):
    x = np.asarray(inputs['x']).astype(np.int64)
    ei = np.asarray(inputs['edge_index']).astype(np.int64)
    batch = np.asarray(inputs['batch']).astype(np.int64)
    emb = np.asarray(inputs['emb'], np.float32)
    W1 = np.asarray(inputs['W1'], np.float32)
    as1 = np.asarray(inputs['a_src1'], np.float32); ad1 = np.asarray(inputs['a_dst1'], np.float32)
    b1 = np.asarray(inputs['b1'], np.float32)
    g1 = np.asarray(inputs['g1'], np.float32); be1 = np.asarray(inputs['be1'], np.float32)
    W2 = np.asarray(inputs['W2'], np.float32)
    as2 = np.asarray(inputs['a_src2'], np.float32); ad2 = np.asarray(inputs['a_dst2'], np.float32)
    b2 = np.asarray(inputs['b2'], np.float32)
    g2 = np.asarray(inputs['g2'], np.float32); be2 = np.asarray(inputs['be2'], np.float32)
    W3 = np.asarray(inputs['W3'], np.float32)
    as3 = np.asarray(inputs['a_src3'], np.float32); ad3 = np.asarray(inputs['a_dst3'], np.float32)
    b3 = np.asarray(inputs['b3'], np.float32)
    g3 = np.asarray(inputs['g3'], np.float32); be3 = np.asarray(inputs['be3'], np.float32)

    NPC = N // NC                      # 6250
    NG = (NPC + 127) // 128            # 49
    NPCP = NG * 128                    # 6272
    CH = NPCP // NCHUNK
    CHF = CH * NC
    NFULL = NC * NPCP                  # 50176
    HALF = NFULL // 2                  # 25088

    # ---- L1 tables (cnt trick) ----
    z1 = emb @ W1                                     # [32, 512]
    z1h = z1.reshape(TILE, H1, C1)
    al1t = np.einsum('thc,hc->th', z1h, as1)          # [32,8]
    ar1t = np.einsum('thc,hc->th', z1h, ad1)
    ee = al1t.T[None, :, :] + ar1t[:, :, None]        # [xd=32, h=8, t=32]
    ee = np.where(ee > 0, ee, 0.2 * ee)
    E_tab = np.exp(ee).astype(np.float32)             # [32, 8, 32]

    src_all = np.concatenate([ei[0], np.arange(N)])
    dst_all = np.concatenate([ei[1], np.arange(N)])
    xs_all = x[src_all]
    cnt = np.zeros((N, TILE), np.float32)
    np.add.at(cnt, (dst_all, xs_all), 1.0)

    # blockdiag Z1B: Z1B[hb][h*32+t, h*64+c] = z1[t, (hb*4+h)*64+c]
    Z1B = np.zeros((2, 128, 256), np.float32)
    for hb in range(2):
        for h in range(4):
            Z1B[hb, h * 32:(h + 1) * 32, h * 64:(h + 1) * 64] = \
                z1[:, (hb * 4 + h) * 64:(hb * 4 + h + 1) * 64]

    def wprime(W, a_s, a_d, H, C):
        As = np.zeros((H * C, H), np.float32)
        Ad = np.zeros((H * C, H), np.float32)
        for h in range(H):
            As[h * C:(h + 1) * C, h] = a_s[h]
            Ad[h * C:(h + 1) * C, h] = a_d[h]
        return np.concatenate([W, W @ As, W @ Ad], axis=1)

    W2p = wprime(W2, as2, ad2, H2, C2)   # [512, 520]
    W3p = wprime(W3, as3, ad3, H3, C3)   # [512, 18]
    W2s = np.concatenate([W2p[k * 128:(k + 1) * 128] for k in range(4)], axis=1)
    W3s = np.concatenate([W3p[k * 128:(k + 1) * 128] for k in range(4)], axis=1)

    def bc(v, F):
        t = np.zeros((128, F), np.float32); t[:, :] = v[None, :F]; return t

    consts = dict(
        Z1B0=Z1B[0].astype(BF), Z1B1=Z1B[1].astype(BF),
        W2s=W2s.astype(BF), W3s=W3s.astype(BF),
        b1t=bc(b1, 512), g1t=bc(g1, 512), be1t=bc(be1, 512),
        b2t=bc(b2, 512), g2t=bc(g2, 512), be2t=bc(be2, 512),
        b3t=bc(b3, 16), g3t=bc(g3, 16), be3t=bc(be3, 16),
        identb=np.eye(128, dtype=BF),
    )

    # ---- per-core edge bucketing ----
    es, ed = ei[0], ei[1]
    c0 = es // NPC
    r = es % NPC
    kk = r // CH
    grow = kk * CHF + c0 * CH + (r - kk * CH)
    half = (grow >= HALF).astype(np.int64)

    core_of = ed // NPC
    rd = ed % NPC
    grp_of = rd // 128
    dstloc = rd % 128

    cnts = np.zeros((NC, NG, 2), np.int64)
    np.add.at(cnts, (core_of, grp_of, half), 1)
    TA = np.maximum(1, (cnts[:, :, 0].max(axis=0) + 127) // 128)
    TB = np.maximum(1, (cnts[:, :, 1].max(axis=0) + 127) // 128)
    Tg = TA + TB
    toff = np.concatenate([[0], np.cumsum(Tg)]).astype(np.int64)
    NTT = int(toff[-1])

    order = np.lexsort((es, half, grp_of, core_of))
    es_s = es[order]; half_s = half[order]
    core_s, grp_s = core_of[order], grp_of[order]
    grow_s = grow[order]; dstloc_s = dstloc[order]
    flat = (core_s * NG + grp_s) * 2 + half_s
    bounds = np.searchsorted(flat, np.arange(NC * NG * 2 + 1))

    percore = []
    for c in range(NC):
        idxg = np.zeros((NTT * 128,), np.int64)
        dlf = np.full((NTT * 128,), 200, np.int64)
        for g in range(NG):
            o = int(toff[g]) * 128
            for h in range(2):
                f = (c * NG + g) * 2 + h
                s, e = bounds[f], bounds[f + 1]
                n = e - s
                oo = o + (int(TA[g]) * 128 if h else 0)
                idxg[oo:oo + n] = grow_s[s:e] - HALF * h
                dlf[oo:oo + n] = dstloc_s[s:e]
        idxg16 = _wrap16(idxg.astype(np.int16))          # [128, NTT*8]
        # S/ST: per group cols [toff[g]*256, (toff[g]+T)*256)
        # layout: [S (T*128) | ST (T*128)] per group
        dlt = dlf.reshape(NTT, 128)                       # [tile, p] -> dstloc
        SST = np.zeros((128, NTT * 256), BF)
        S_t = (dlt[:, :, None] == np.arange(128)[None, None, :])  # [NTT,p,d]
        for g in range(NG):
            t0, T = int(toff[g]), int(Tg[g])
            blk = S_t[t0:t0 + T]                          # [T, p, d]
            SST[:, t0 * 256:t0 * 256 + T * 128] = \
                blk.transpose(1, 0, 2).reshape(128, T * 128).astype(BF)
            SST[:, t0 * 256 + T * 128:(t0 + T) * 256] = \
                blk.transpose(2, 0, 1).reshape(128, T * 128).astype(BF)

        lo, hi = c * NPC, (c + 1) * NPC
        cntc = np.zeros((NPCP, TILE), BF)
        cntc[:NPC] = cnt[lo:hi].astype(BF)
        cntc[NPC:, 0] = 1.0
        Ec = np.zeros((NPCP, H1 * TILE), BF)
        Ec[:NPC] = E_tab[x[lo:hi]].reshape(NPC, H1 * TILE).astype(BF)
        Ec[NPC:] = 1.0
        OBc = np.zeros((NPCP, G), BF)
        OBc[np.arange(NPC), batch[lo:hi]] = 1.0
        percore.append(dict(cntc=cntc, Ec=Ec, OBc=OBc, idxg16=idxg16, SST=SST))

    meta = dict(N=N, E=E, G=G, NC=NC, NPC=NPC, NG=NG, NPCP=NPCP,
                CH=CH, CHF=CHF, NFULL=NFULL, HALF=HALF,
                TA=TA.tolist(), TB=TB.tolist(), Tg=Tg.tolist(),
                toff=toff.tolist())
    host = dict(fcW1=np.asarray(inputs['fcW1'], np.float32),
                fcb1=np.asarray(inputs['fcb1'], np.float32),
                fcW2=np.asarray(inputs['fcW2'], np.float32),
                fcb2=np.asarray(inputs['fcb2'], np.float32))
    return consts, percore, meta, host


def emit_gather(nc, out3, table, idxt, ntiles, elem, elem_step=None, t0=0, i0=0):
    """dma_gather capped at 1024 idxs (8 tiles) per instruction, round-robin
    across SWDGE queues so descriptor-gen overlaps DMA drains."""
    CAP = 8
    t = 0
    while t < ntiles:
        n = min(CAP, ntiles - t)
        q = _QCTR[0] % NQ
        _QCTR[0] += 1
        nc.gpsimd.dma_gather(
            out3[:, t0 + t:t0 + t + n, :], table,
            idxt[:, (i0 + t) * 8:(i0 + t + n) * 8],
            n * 128, n * 128, elem, elem_step=elem_step, queue_num=q)
        t += n


def layer_norm_elu(nc, pool, y, g_t, be_t, F, epsc, out_dtype=BF16, tag=""):
    """In SBUF f32: y [128,F] -> elu(LN(y)*g+be) in out_dtype. New tile."""
    s1 = pool.tile([128, 1], F32, tag="ln_s1" + tag)
    nc.vector.tensor_reduce(out=s1[:], in_=y[:], axis=AX.X, op=ALU.add)
    m2 = pool.tile([128, 1], F32, tag="ln_m2" + tag)
    nc.vector.tensor_scalar_mul(out=m2[:], in0=s1[:], scalar1=-1.0 / F)
    sq = pool.tile([128, F], F32, tag="ln_sq" + tag)
    ss = pool.tile([128, 1], F32, tag="ln_ss" + tag)
    nc.scalar.activation(out=sq[:], in_=y[:], func=ACT.Square, bias=m2[:, :1],
                         accum_out=ss[:])
    sd = pool.tile([128, 1], F32, tag="ln_sd" + tag)
    nc.scalar.activation(out=sd[:], in_=ss[:], func=ACT.Sqrt, bias=epsc[:, :1], scale=1.0 / F)
    rs = pool.tile([128, 1], F32, tag="ln_rs" + tag)
    nc.vector.reciprocal(out=rs[:], in_=sd[:])
    nc.vector.tensor_scalar(out=y[:], in0=y[:], scalar1=m2[:, :1], scalar2=rs[:, :1],
                            op0=ALU.add, op1=ALU.mult)
    nc.vector.tensor_tensor(out=y[:], in0=y[:], in1=g_t[:, :F], op=ALU.mult)
    nc.vector.tensor_tensor(out=y[:], in0=y[:], in1=be_t[:, :F], op=ALU.add)
    nc.vector.tensor_scalar_min(out=sq[:], in0=y[:], scalar1=0.0)
    nc.scalar.activation(out=sq[:], in_=sq[:], func=ACT.Exp)
    h = pool.tile([128, F], out_dtype, tag="elu_h" + tag)
    nc.vector.tensor_scalar(out=y[:], in0=y[:], scalar1=0.0, scalar2=-1.0,
                            op0=ALU.max, op1=ALU.add)
    nc.vector.tensor_tensor(out=h[:], in0=y[:], in1=sq[:], op=ALU.add)
    return h


def build(meta):
    NC, NG, NPCP = meta['NC'], meta['NG'], meta['NPCP']
    TA, TB, Tg, toff = meta['TA'], meta['TB'], meta['Tg'], meta['toff']
    NTT = toff[-1]
    CH, NFULL, HALF = meta['CH'], meta['NFULL'], meta['HALF']
    CHF = meta['CHF']
    G = meta['G']

    nc = bacc.Bacc("TRN2", num_devices=NC, num_swdge_queues=NQ)
    _QCTR[0] = 0
    t_cnt = nc.dram_tensor("cntc", [NPCP, TILE], BF16, kind="ExternalInput")
    t_E = nc.dram_tensor("Ec", [NPCP, H1 * TILE], BF16, kind="ExternalInput")
    t_OB = nc.dram_tensor("OBc", [NPCP, G], BF16, kind="ExternalInput")
    t_ig = nc.dram_tensor("idxg16", [128, NTT * 8], I16, kind="ExternalInput")
    t_SST = nc.dram_tensor("SST", [128, NTT * 256], BF16, kind="ExternalInput")
    t_Z0 = nc.dram_tensor("Z1B0", [128, 256], BF16, kind="ExternalInput")
    t_Z1 = nc.dram_tensor("Z1B1", [128, 256], BF16, kind="ExternalInput")
    t_W2s = nc.dram_tensor("W2s", [128, 4 * 520], BF16, kind="ExternalInput")
    t_W3s = nc.dram_tensor("W3s", [128, 4 * 18], BF16, kind="ExternalInput")
    cn = {}
    for nm, sh in [("b1t", 512), ("g1t", 512), ("be1t", 512), ("b2t", 512),
                   ("g2t", 512), ("be2t", 512), ("b3t", 16), ("g3t", 16), ("be3t", 16)]:
        cn[nm] = nc.dram_tensor(nm, [128, sh], F32, kind="ExternalInput")
    t_idb = nc.dram_tensor("identb", [128, 128], BF16, kind="ExternalInput")
    t_out = nc.dram_tensor("part", [G, 17], F32, kind="ExternalOutput")

    with tile.TileContext(nc) as tc:
        with tc.tile_pool(name="const", bufs=1) as cp, \
             tc.tile_pool(name="sb", bufs=2) as sb, \
             tc.tile_pool(name="gbuf", bufs=3) as gb, \
             tc.tile_pool(name="ps", bufs=1, space="PSUM") as ps, \
             tc.tile_pool(name="pst", bufs=2, space="PSUM") as pst, \
             tc.tile_pool(name="pacc", bufs=1, space="PSUM") as pacc, \
             tc.tile_pool(name="dram", bufs=1, space="DRAM") as dp:

            C = {}
            for nm, src, shp, dt in [
                    ("identb", t_idb, [128, 128], BF16),
                    ("Z1B0", t_Z0, [128, 256], BF16),
                    ("Z1B1", t_Z1, [128, 256], BF16),
                    ("W2s", t_W2s, [128, 4 * 520], BF16),
                    ("W3s", t_W3s, [128, 4 * 18], BF16)]:
                C[nm] = cp.tile(shp, dt, tag="c_" + nm, name="c_" + nm)
                nc.sync.dma_start(out=C[nm][:], in_=src[:])
            for nm in cn:
                F = 512 if nm[-2] != '3' else 16
                C[nm] = cp.tile([128, F], F32, tag="c_" + nm, name="c_" + nm)
                nc.sync.dma_start(out=C[nm][:], in_=cn[nm][:])
            epsc = cp.tile([128, 1], F32, name="epsc")
            nc.vector.memset(epsc[:], EPS)

            rec2_sh = dp.tile([NPCP, REC2], BF16)
            rec2_full = dp.tile([NFULL, REC2], BF16, addr_space="Shared")
            rec3_sh = dp.tile([NPCP, REC3], BF16)
            rec3_full = dp.tile([NFULL, REC3], BF16, addr_space="Shared")

            GPC = NG // NCHUNK

            # ================= L1 + phaseA(L2) =================
            for g in range(NG):
                r0 = g * 128
                cg = sb.tile([128, TILE], BF16, tag="cg")
                nc.sync.dma_start(out=cg[:], in_=t_cnt[r0:r0 + 128, :])
                Eg = sb.tile([128, H1, TILE], BF16, tag="Eg")
                nc.sync.dma_start(out=Eg[:, :, :], in_=t_E[r0:r0 + 128, :].rearrange("p (h t) -> p h t", h=H1))
                M = sb.tile([128, H1, TILE], BF16, tag="M")
                nc.vector.tensor_tensor(out=M[:, :, :], in0=Eg[:, :, :],
                                        in1=cg[:, None, :].to_broadcast([128, H1, TILE]),
                                        op=ALU.mult)
                s = sb.tile([128, H1], F32, tag="s")
                nc.vector.tensor_reduce(out=s[:], in_=M[:, :, :], axis=AX.X, op=ALU.add)
                rs = sb.tile([128, H1], F32, tag="rs")
                nc.vector.reciprocal(out=rs[:], in_=s[:])
                Pb = sb.tile([128, H1, TILE], BF16, tag="Pb")
                nc.vector.tensor_tensor(out=Pb[:, :, :], in0=M[:, :, :],
                                        in1=rs[:, :, None].to_broadcast([128, H1, TILE]),
                                        op=ALU.mult)
                Pf = Pb[:].rearrange("p h t -> p (h t)")
                pO = ps.tile([128, 512], F32, tag="pacc_main", space="PSUM")
                for hb in range(2):
                    ptp = pst.tile([128, 128], BF16, tag="tp_ps", space="PSUM")
                    nc.tensor.transpose(out=ptp[:], in_=Pf[:, hb * 128:(hb + 1) * 128],
                                        identity=C["identb"][:])
                    PT = sb.tile([128, 128], BF16, tag="PT")
                    nc.vector.tensor_copy(out=PT[:], in_=ptp[:])
                    nc.tensor.matmul(out=pO[:, hb * 256:(hb + 1) * 256], lhsT=PT[:],
                                     rhs=C["Z1B0" if hb == 0 else "Z1B1"][:],
                                     start=True, stop=True)
                y = sb.tile([128, 512], F32, tag="y1")
                nc.vector.tensor_tensor(out=y[:], in0=pO[:], in1=C["b1t"][:], op=ALU.add)
                h1 = layer_norm_elu(nc, sb, y, C["g1t"], C["be1t"], 512, epsc)
                pz = ps.tile([128, 512], F32, tag="pz", space="PSUM")
                pzb = ps.tile([128, 8], F32, tag="pzb", space="PSUM")
                for k in range(4):
                    ptp = pst.tile([128, 128], BF16, tag="tp_ps", space="PSUM")
                    nc.tensor.transpose(out=ptp[:], in_=h1[:, k * 128:(k + 1) * 128],
                                        identity=C["identb"][:])
                    hT = sb.tile([128, 128], BF16, tag="hT")
                    nc.vector.tensor_copy(out=hT[:], in_=ptp[:])
                    nc.tensor.matmul(out=pz[:], lhsT=hT[:], rhs=C["W2s"][:, k * 520:k * 520 + 512],
                                     start=(k == 0), stop=(k == 3))
                    nc.tensor.matmul(out=pzb[:], lhsT=hT[:], rhs=C["W2s"][:, k * 520 + 512:(k + 1) * 520],
                                     start=(k == 0), stop=(k == 3))
                zs = sb.tile([128, REC2], BF16, tag="zs")
                nc.scalar.activation(out=zs[:, :512], in_=pz[:], func=ACT.Copy)
                nc.scalar.activation(out=zs[:, 512:520], in_=pzb[:], func=ACT.Copy)
                nc.vector.memset(zs[:, 520:], 0.0)
                nc.scalar.dma_start(out=rec2_sh[r0:r0 + 128, :], in_=zs[:])
                if (g + 1) % GPC == 0:
                    k = g // GPC
                    nc.gpsimd.collective_compute(
                        "AllGather", ALU.bypass, replica_groups=[list(range(NC))],
                        ins=[rec2_sh[k * CH:(k + 1) * CH, :].opt()],
                        outs=[rec2_full[k * CHF:(k + 1) * CHF, :].opt()])

            # ================= L2 + phaseA(L3) =================
            for g in range(NG):
                r0 = g * 128
                T = Tg[g]; tA = TA[g]; tB = TB[g]
                o0 = toff[g]
                ia = sb.tile([128, T * 8], I16, tag="ia")
                nc.sync.dma_start(out=ia[:], in_=t_ig[:, o0 * 8:(o0 + T) * 8])
                SST = gb.tile([128, 2 * T, 128], BF16, tag="SST")
                nc.sync.dma_start(out=SST[:].rearrange("p t d -> p (t d)"),
                                  in_=t_SST[:, o0 * 256:(o0 + T) * 256])
                zg = sb.tile([128, 520], BF16, tag="zg")
                nc.scalar.dma_start(out=zg[:], in_=rec2_sh[r0:r0 + 128, :520])
                Gt = gb.tile([128, T, REC2], BF16, tag="G")
                emit_gather(nc, Gt, rec2_full[0:HALF, :], ia, tA, REC2)
                emit_gather(nc, Gt, rec2_full[HALF:NFULL, :], ia, tB, REC2,
                            t0=tA, i0=tA)
                # per-edge ar[dst] via ST_t^T @ zgar
                pAR = ps.tile([128, T * H2], F32, tag="pAR", space="PSUM")
                for t in range(T):
                    nc.tensor.matmul(out=pAR[:, t * H2:(t + 1) * H2],
                                     lhsT=SST[:, T + t, :], rhs=zg[:, 516:520],
                                     start=True, stop=True)
                eL = sb.tile([128, T * H2], F32, tag="eL")
                nc.vector.tensor_tensor(
                    out=eL[:].rearrange("p (t h) -> p t h", h=H2),
                    in0=Gt[:, :, 512:516], in1=pAR[:].rearrange("p (t h) -> p t h", h=H2),
                    op=ALU.add)
                eA = sb.tile([128, T * H2], F32, tag="eA")
                nc.vector.tensor_scalar_mul(out=eA[:], in0=eL[:], scalar1=0.2)
                nc.vector.tensor_tensor(out=eA[:], in0=eL[:], in1=eA[:], op=ALU.max)
                EX = sb.tile([128, T * H2], BF16, tag="EX")
                nc.scalar.activation(out=EX[:], in_=eA[:], func=ACT.Exp)
                nc.vector.tensor_tensor(
                    out=Gt[:, :, :512].rearrange("p t (h c) -> p t h c", h=H2),
                    in0=Gt[:, :, :512].rearrange("p t (h c) -> p t h c", h=H2),
                    in1=EX[:].rearrange("p (t h) -> p t h", h=H2)[:, :, :, None]
                        .to_broadcast([128, T, H2, C2]),
                    op=ALU.mult)
                pMain = ps.tile([128, 512], F32, tag="pacc_main", space="PSUM")
                pS = ps.tile([128, H2], F32, tag="pacc_s", space="PSUM")
                for t in range(T):
                    nc.tensor.matmul(out=pMain[:], lhsT=SST[:, t, :], rhs=Gt[:, t, :512],
                                     start=(t == 0), stop=(t == T - 1))
                    nc.tensor.matmul(out=pS[:], lhsT=SST[:, t, :], rhs=EX[:, t * H2:(t + 1) * H2],
                                     start=(t == 0), stop=(t == T - 1))
                # self-loop (clamp at 30: pad-row records can be large)
                eSl = sb.tile([128, H2], F32, tag="eSl")
                nc.vector.tensor_tensor(out=eSl[:], in0=zg[:, 512:516], in1=zg[:, 516:520], op=ALU.add)
                nc.vector.tensor_scalar_min(out=eSl[:], in0=eSl[:], scalar1=30.0)
                eSa = sb.tile([128, H2], F32, tag="eSa")
                nc.vector.tensor_scalar_mul(out=eSa[:], in0=eSl[:], scalar1=0.2)
                nc.vector.tensor_tensor(out=eSa[:], in0=eSl[:], in1=eSa[:], op=ALU.max)
                exS = sb.tile([128, H2], F32, tag="exS")
                nc.scalar.activation(out=exS[:], in_=eSa[:], func=ACT.Exp)
                selfc = sb.tile([128, 512], F32, tag="selfc")
                nc.vector.tensor_tensor(
                    out=selfc[:].rearrange("p (h c) -> p h c", h=H2),
                    in0=zg[:, :512].rearrange("p (h c) -> p h c", h=H2),
                    in1=exS[:, :, None].to_broadcast([128, H2, C2]), op=ALU.mult)
                nc.vector.tensor_tensor(out=selfc[:], in0=pMain[:], in1=selfc[:], op=ALU.add)
                sS = sb.tile([128, H2], F32, tag="sS")
                nc.vector.tensor_tensor(out=sS[:], in0=pS[:], in1=exS[:], op=ALU.add)
                rS = sb.tile([128, H2], F32, tag="rS")
                nc.vector.reciprocal(out=rS[:], in_=sS[:])
                nc.vector.tensor_tensor(
                    out=selfc[:].rearrange("p (h c) -> p h c", h=H2),
                    in0=selfc[:].rearrange("p (h c) -> p h c", h=H2),
                    in1=rS[:, :, None].to_broadcast([128, H2, C2]), op=ALU.mult)
                nc.vector.tensor_tensor(out=selfc[:], in0=selfc[:], in1=C["b2t"][:], op=ALU.add)
                h2 = layer_norm_elu(nc, sb, selfc, C["g2t"], C["be2t"], 512, epsc)
                pz3 = ps.tile([128, 18], F32, tag="pz", space="PSUM")
                for k in range(4):
                    ptp = pst.tile([128, 128], BF16, tag="tp_ps", space="PSUM")
                    nc.tensor.transpose(out=ptp[:], in_=h2[:, k * 128:(k + 1) * 128],
                                        identity=C["identb"][:])
                    hT = sb.tile([128, 128], BF16, tag="hT")
                    nc.vector.tensor_copy(out=hT[:], in_=ptp[:])
                    nc.tensor.matmul(out=pz3[:], lhsT=hT[:], rhs=C["W3s"][:, k * 18:(k + 1) * 18],
                                     start=(k == 0), stop=(k == 3))
                z3s = sb.tile([128, REC3], BF16, tag="z3s")
                nc.scalar.activation(out=z3s[:, :18], in_=pz3[:], func=ACT.Copy)
                nc.vector.memset(z3s[:, 18:], 0.0)
                nc.scalar.dma_start(out=rec3_sh[r0:r0 + 128, :], in_=z3s[:])
                if (g + 1) % GPC == 0:
                    k = g // GPC
                    nc.gpsimd.collective_compute(
                        "AllGather", ALU.bypass, replica_groups=[list(range(NC))],
                        ins=[rec3_sh[k * CH:(k + 1) * CH, :].opt()],
                        outs=[rec3_full[k * CHF:(k + 1) * CHF, :].opt()])

            # ================= L3 + pooling =================
            pPool = pacc.tile([128, 17], F32, tag="pPool", space="PSUM")
            for g in range(NG):
                r0 = g * 128
                T = Tg[g]; tA = TA[g]; tB = TB[g]
                o0 = toff[g]
                ia = sb.tile([128, T * 8], I16, tag="ia")
                nc.sync.dma_start(out=ia[:], in_=t_ig[:, o0 * 8:(o0 + T) * 8])
                SST = gb.tile([128, 2 * T, 128], BF16, tag="SST")
                nc.sync.dma_start(out=SST[:].rearrange("p t d -> p (t d)"),
                                  in_=t_SST[:, o0 * 256:(o0 + T) * 256])
                zg3 = sb.tile([128, 18], BF16, tag="zg3")
                nc.scalar.dma_start(out=zg3[:], in_=rec3_sh[r0:r0 + 128, :18])
                OB = sb.tile([128, G], BF16, tag="OB")
                nc.sync.dma_start(out=OB[:], in_=t_OB[r0:r0 + 128, :])
                Gt = gb.tile([128, T, REC3], BF16, tag="G3")
                emit_gather(nc, Gt, rec3_full[0:HALF, :], ia, tA, REC3)
                emit_gather(nc, Gt, rec3_full[HALF:NFULL, :], ia, tB, REC3,
                            t0=tA, i0=tA)
                pAR = ps.tile([128, T], F32, tag="pAR", space="PSUM")
                for t in range(T):
                    nc.tensor.matmul(out=pAR[:, t:t + 1],
                                     lhsT=SST[:, T + t, :], rhs=zg3[:, 17:18],
                                     start=True, stop=True)
                eL = sb.tile([128, T], F32, tag="eL3")
                nc.vector.tensor_tensor(out=eL[:], in0=Gt[:, :, 16], in1=pAR[:], op=ALU.add)
                eA = sb.tile([128, T], F32, tag="eA3")
                nc.vector.tensor_scalar_mul(out=eA[:], in0=eL[:], scalar1=0.2)
                nc.vector.tensor_tensor(out=eA[:], in0=eL[:], in1=eA[:], op=ALU.max)
                EX = sb.tile([128, T], BF16, tag="EX3")
                nc.scalar.activation(out=EX[:], in_=eA[:], func=ACT.Exp)
                nc.vector.tensor_tensor(
                    out=Gt[:, :, :16], in0=Gt[:, :, :16],
                    in1=EX[:, :, None].to_broadcast([128, T, 16]), op=ALU.mult)
                nc.vector.tensor_copy(out=Gt[:, :, 16:17],
                                      in_=EX[:].rearrange("p (t o) -> p t o", o=1))
                pF = ps.tile([128, 17], F32, tag="pacc_main", space="PSUM")
                for t in range(T):
                    nc.tensor.matmul(out=pF[:], lhsT=SST[:, t, :], rhs=Gt[:, t, :17],
                                     start=(t == 0), stop=(t == T - 1))
                eSl = sb.tile([128, 1], F32, tag="eSl3")
                nc.vector.tensor_tensor(out=eSl[:], in0=zg3[:, 16:17], in1=zg3[:, 17:18], op=ALU.add)
                nc.vector.tensor_scalar_min(out=eSl[:], in0=eSl[:], scalar1=30.0)
                eSa = sb.tile([128, 1], F32, tag="eSa3")
                nc.vector.tensor_scalar_mul(out=eSa[:], in0=eSl[:], scalar1=0.2)
                nc.vector.tensor_tensor(out=eSa[:], in0=eSl[:], in1=eSa[:], op=ALU.max)
                exS = sb.tile([128, 1], F32, tag="exS3")
                nc.scalar.activation(out=exS[:], in_=eSa[:], func=ACT.Exp)
                selfc = sb.tile([128, 16], F32, tag="selfc3")
                nc.vector.tensor_scalar(out=selfc[:], in0=zg3[:, :16], scalar1=exS[:, :1],
                                        scalar2=None, op0=ALU.mult)
                nc.vector.tensor_tensor(out=selfc[:], in0=pF[:, :16], in1=selfc[:], op=ALU.add)
                sS = sb.tile([128, 1], F32, tag="sS3")
                nc.vector.tensor_tensor(out=sS[:], in0=pF[:, 16:17], in1=exS[:], op=ALU.add)
                rS = sb.tile([128, 1], F32, tag="rS3")
                nc.vector.reciprocal(out=rS[:], in_=sS[:])
                nc.vector.tensor_scalar(out=selfc[:], in0=selfc[:], scalar1=rS[:, :1],
                                        scalar2=None, op0=ALU.mult)
                nc.vector.tensor_tensor(out=selfc[:], in0=selfc[:], in1=C["b3t"][:], op=ALU.add)
                h3 = layer_norm_elu(nc, sb, selfc, C["g3t"], C["be3t"], 16, epsc, tag="3")
                h3w = sb.tile([128, 17], BF16, tag="h3w")
                nc.vector.tensor_copy(out=h3w[:, :16], in_=h3[:])
                nc.vector.memset(h3w[:, 16:17], 1.0)
                nc.tensor.matmul(out=pPool[:G, :17], lhsT=OB[:], rhs=h3w[:],
                                 start=(g == 0), stop=(g == NG - 1))
            po = sb.tile([128, 17], F32, tag="po")
            nc.vector.tensor_copy(out=po[:G, :], in_=pPool[:G, :])
            nc.sync.dma_start(out=t_out[:, :], in_=po[:G, :])
    nc.finalize()
    return nc


# ======================= kernel entry =======================
_CACHE = {}


def _cache_key(meta):
    return (tuple(meta['TA']), tuple(meta['TB']))


def _hw_runner(nc, in_maps):
    from concourse.bass_utils import run_bass_kernel_spmd
    res = run_bass_kernel_spmd(nc, in_maps, core_ids=list(range(len(in_maps))))
    return res.results


def kernel(**inputs):
    consts, percore, meta, host = host_prep(inputs, N_FULL, E_FULL, G_FULL, NC_FULL)
    key = _cache_key(meta)
    if key not in _CACHE:
        _CACHE[key] = build(meta)
    nc = _CACHE[key]
    in_maps = []
    for c in range(NC_FULL):
        m = dict(consts)
        m.update(percore[c])
        in_maps.append(m)
    results = _hw_runner(nc, in_maps)
    parts = np.stack([r["part"] for r in results])
    tot = parts.sum(axis=0)
    pooled = tot[:, :16] / np.maximum(tot[:, 16:17], 1.0)
    h = np.maximum(pooled @ host['fcW1'] + host['fcb1'], 0.0)
    return (h @ host['fcW2'] + host['fcb2']).astype(np.float32)
